# revision 24
# baseline (speedup 1.0000x reference)
"""Causal multi-head attention layer for Trainium2, sharded over 8 NeuronCores.

Problem: B=2, T=2048, E=1024, H=16 heads (D=64), fp32.
  out = softmax(mask(QK^T)/sqrt(E)) V Wo + bo   with Q=xWq+bq etc.

Sharding: data-parallel over batch (2) x tensor-parallel over head groups (4):
core c -> batch b=c//4, head group g=c%4 (4 heads, 256 channels).
Each core computes partial = attn_heads(x_b) @ Wo[rows of g]; host sums the 4
partials per batch and adds the bias row.

Math folds used (all exact):
 - bk drops out of softmax (additive shift along the softmax axis).
 - attn rows sum to 1  =>  attn @ (V + 1 bv^T) = attn@V + 1 bv^T, so bv enters
   the output as the constant row bv @ Wo, added on the host with bo.
 - bq is added to Q^T on-chip (per-partition bias).

Measured PE law (trace): effective clock duty-cycles ~1.2-2.4GHz (power
throttle), bf16/fp8 stream 1 col/cycle, fp8e4 DoubleRow streams 2 phys cols
per cycle for a 256-deep contraction. So DR fp8 ~2x bf16 throughput; plain
fp8 = bf16 speed.

Precision: fp8 error concentrates in rows whose softmax averages few keys
(early rows). Chunk 0 (q<512) keeps a bf16 V + bf16 diag path; chunks 1-3
run the causal-diagonal kt-pairs in fp8 DR like the off-diagonal (those rows
attend >=512 keys). V is projected in fp8 DR for key tiles 4..15 and bf16 for
tiles 0..3.

Device layout per core (matmuls: out = lhsT.T @ rhs, contraction on
partitions):
 - Q^T/K^T = W^T x^T in fp8 DoubleRow over e-tile pairs -> bf16 [ch, t].
 - V tiles 0..3: bf16 x W -> vo_bf [k, h, D+1] (ones col for the denominator)
   + fp8 cast into vop; tiles 4..15: fp8 DR -> vop [k, ktpair, h, 128-pad]
   directly (ones row at 64, zero pad above).
 - energy^T = K Q^T per head, bf16, d=64 contraction at PE row base 0/64
   (64x128 row tiling runs the two heads of a pair concurrently); causal
   mask added (-1e9) on PSUM before exp.
 - exp on ACT: chunk 0 diag pairs -> bf16 U; everything else -> fp8 U (diag
   pairs split j0/j1 + gpsimd memset of the masked gap so DR sees zeros).
 - O^T accumulates per head in PSUM: one fp8 DR matmul per kt-pair (V rows
   0..63, ones denominator row 64, zero pad above); chunk 0 diag in bf16
   M=65 matmuls.
 - normalize with DVE mult by gpsimd-broadcast reciprocal; the final pair
   uses a PE outer-product broadcast (fp32r) instead so the tail chain is
   short; odd heads shift to partitions 64..127 via SBUF->SBUF DMA.
 - partial = (O^T).T @ Wo rows in bf16, PSUM -> SBUF -> DRAM. The last
   chunk's Wo is split: m=0 half staged to SBUF during the final pair's
   attention, m=1 half + DVE add + DMA after the last norm.
"""

import os
import numpy as np

B, T, E, H = 2, 2048, 1024, 16
P = 128
NCORES = 8
G = 4            # head groups (tensor parallel)
HG = H // G      # heads per group = 4
D = E // H       # 64
CW = HG * D      # channels per group = 256
ET = E // P      # 8 e-tiles
AP2 = ET // 2    # 4 e-tile pairs
MT = CW // P     # 2 hd-tiles
TT = T // P      # 16 t-tiles
NQ = 512         # q-chunk width
QC = T // NQ     # 4 q-chunks
NEG = -1.0e9
SCALE = 1.0 / np.sqrt(E).astype(np.float32)  # 1/32

PH2 = True           # fp8-DR V (tiles>=4) + fp8-DR diag AV (qc>=1)

_CACHE: dict = {}


def _build_bass(debug_taps=False):
    import concourse.bass as bass
    import concourse.mybir as mybir
    import concourse.tile as tile
    from concourse import bacc

    f32 = mybir.dt.float32
    bf16 = mybir.dt.bfloat16
    f8 = mybir.dt.float8e4
    DR = mybir.MatmulPerfMode.DoubleRow
    Exp = mybir.ActivationFunctionType.Exp

    nc = bacc.Bacc("TRN2", target_bir_lowering=False, name="attn_tp")
    dbg = {}
    if debug_taps:
        dbg["qt00"] = nc.dram_tensor("dbg_qt00", [P, NQ], bf16, kind="ExternalOutput")
        dbg["kt00"] = nc.dram_tensor("dbg_kt00", [P, NQ], bf16, kind="ExternalOutput")
        dbg["vo0"] = nc.dram_tensor("dbg_vo0", [P, HG * (D + 1)], bf16, kind="ExternalOutput")
        dbg["u000"] = nc.dram_tensor("dbg_u000", [P, 2 * NQ], bf16, kind="ExternalOutput")
        dbg["ot00"] = nc.dram_tensor("dbg_ot00", [P, NQ], bf16, kind="ExternalOutput")
    XTBW = NQ if PH2 else T
    xt = nc.dram_tensor("xt", [P, ET, T], f8, kind="ExternalInput")
    xtb = nc.dram_tensor("xtb", [P, ET, XTBW], bf16, kind="ExternalInput")
    wq = nc.dram_tensor("wq", [P, ET, CW], f8, kind="ExternalInput")
    wk = nc.dram_tensor("wk", [P, ET, CW], f8, kind="ExternalInput")
    wv = nc.dram_tensor("wv", [P, ET, CW], bf16, kind="ExternalInput")
    wv8 = nc.dram_tensor("wv8", [P, ET, CW], f8, kind="ExternalInput")
    bq = nc.dram_tensor("bq", [P, MT], f32, kind="ExternalInput")
    wo = nc.dram_tensor("wo", [P, MT, E], bf16, kind="ExternalInput")
    tn = nc.dram_tensor("tn", [P, P], f32, kind="ExternalInput")
    out = nc.dram_tensor("out", [T, E], f32, kind="ExternalOutput")

    NVB = 4 if PH2 else TT   # number of bf16 V tiles

    with tile.TileContext(nc) as tc:
        with (
            tc.tile_pool(name="persist", bufs=1) as pers,
            tc.tile_pool(name="pp", bufs=2, space="PSUM") as pp,
            tc.tile_pool(name="ep", bufs=2, space="PSUM") as ep,
            tc.tile_pool(name="op", bufs=2, space="PSUM") as op,
            tc.tile_pool(name="up", bufs=6) as up,
            tc.tile_pool(name="ub", bufs=4) as ub,
            tc.tile_pool(name="sm", bufs=4) as sm,
            tc.tile_pool(name="ost", bufs=6) as ost,
        ):
            # ---- persistent SBUF tensors ----
            xt_sb = pers.tile([P, ET, T], f8, tag="xt_sb", name="xt_sb")
            xtb_sb = pers.tile([P, ET, XTBW], bf16, tag="xtb_sb", name="xtb_sb")
            wq_sb = pers.tile([P, ET, CW], f8, tag="wq_sb", name="wq_sb")
            wk_sb = pers.tile([P, ET, CW], f8, tag="wk_sb", name="wk_sb")
            wv_sb = pers.tile([P, ET, CW], bf16, tag="wv_sb", name="wv_sb")
            wv8_sb = pers.tile([P, ET, CW], f8, tag="wv8_sb", name="wv8_sb")
            wo_sb = pers.tile([P, MT, E], bf16, tag="wo_sb", name="wo_sb")
            bq_sb = pers.tile([P, MT], f32, tag="bq_sb", name="bq_sb")
            tn_sb = pers.tile([P, P], f32, tag="tn_sb", name="tn_sb")
            qt_t = [[pers.tile([P, NQ], bf16, tag=f"qt{m}_{n}", name=f"qt{m}_{n}")
                     for n in range(QC)] for m in range(MT)]
            kt_t = [[pers.tile([P, NQ], bf16, tag=f"kt{m}_{n}", name=f"kt{m}_{n}")
                     for n in range(QC)] for m in range(MT)]
            vo_bf = [pers.tile([P, HG, P], bf16, tag=f"vb{t}", name=f"vb{t}")
                     for t in range(NVB)]
            vop_t = [pers.tile([P, 2, HG, P], f8, tag=f"vo{tp}", name=f"vo{tp}")
                     for tp in range(TT // 2)]
            ot_t = [[pers.tile([P, NQ], bf16, tag=f"ot{m}_{n}", name=f"ot{m}_{n}")
                     for n in range(QC)] for m in range(MT)]
            wo_stage = [pers.tile([P, NQ], f32, tag=f"ws{g}", name=f"ws{g}")
                        for g in range(8)]

            # stationary V layout per (kt, head): parity-split so the AV
            # output lands where the ot tile wants it with no partition
            # shift: even heads [V 0:64 | ones@64 | zeros], odd heads
            # [ones@0 | zeros | V 64:128]. The ones row makes the DR matmul
            # emit the softmax denominator (even: PSUM row 64, odd: row 0).
            # Static ones/zero regions are initialized once on gpsimd.
            for tp in range(TT // 2):
                nc.gpsimd.memset(vop_t[tp][:, :, 0::2, D:D + 1], 1.0)
                nc.gpsimd.memset(vop_t[tp][:, :, 0::2, D + 1:], 0.0)
                nc.gpsimd.memset(vop_t[tp][:, :, 1::2, 0:1], 1.0)
                nc.gpsimd.memset(vop_t[tp][:, :, 1::2, 1:D], 0.0)
            for t in range(NVB):
                nc.gpsimd.memset(vo_bf[t][:, 0::2, D:D + 1], 1.0)
                nc.gpsimd.memset(vo_bf[t][:, 0::2, D + 1:], 0.0)
                nc.gpsimd.memset(vo_bf[t][:, 1::2, 0:1], 1.0)
                nc.gpsimd.memset(vo_bf[t][:, 1::2, 1:D], 0.0)

            # ---- input DMAs ----
            # weights first so the first projection starts early; x arrives
            # in column chunks so chunk-n projections do not wait on full x
            nc.scalar.dma_start(out=wq_sb, in_=wq[:, :, :])
            nc.scalar.dma_start(out=wk_sb, in_=wk[:, :, :])
            nc.scalar.dma_start(out=bq_sb, in_=bq[:, :])
            # chunk-0 columns split by e-tile pair so the first projection
            # matmul starts after the first quarter arrives
            for i in range(AP2):
                nc.sync.dma_start(out=xt_sb[:, 2 * i:2 * i + 2, 0:NQ],
                                  in_=xt[:, 2 * i:2 * i + 2, 0:NQ])
            nc.scalar.dma_start(out=wv_sb, in_=wv[:, :, :])
            nc.scalar.dma_start(out=wv8_sb, in_=wv8[:, :, :])
            nc.sync.dma_start(out=xtb_sb[:, :, 0:NQ], in_=xtb[:, :, 0:NQ])
            for n in range(1, QC):
                nc.sync.dma_start(out=xt_sb[:, :, n * NQ:(n + 1) * NQ],
                                  in_=xt[:, :, n * NQ:(n + 1) * NQ])
                if not PH2 and n < QC:
                    nc.scalar.dma_start(out=xtb_sb[:, :, n * NQ:(n + 1) * NQ],
                                        in_=xtb[:, :, n * NQ:(n + 1) * NQ])
            nc.scalar.dma_start(out=tn_sb, in_=tn[:, :])
            nc.scalar.dma_start(out=wo_sb, in_=wo[:, :, :])

            # ---- software-pipelined emission ----
            # PE engine queues are in-order, so attention batches (gated on
            # ACT exp) are interleaved with independent filler work: the next
            # chunk's projection groups and the previous chunk's Wo groups.

            def proj_closures(n):
                def qk_group(wsb, dst, m, biased):
                    def f():
                        ps = pp.tile([P, NQ], f32, tag="pp_t", name="psqk")
                        for i in range(AP2):
                            nc.tensor.matmul(
                                ps,
                                lhsT=wsb[:, 2 * i:2 * i + 2, m * P:(m + 1) * P],
                                rhs=xt_sb[:, 2 * i:2 * i + 2,
                                          n * NQ:(n + 1) * NQ],
                                start=(i == 0), stop=(i == AP2 - 1),
                                perf_mode=DR,
                            )
                        if biased:
                            nc.vector.tensor_scalar_add(
                                out=dst[m][n], in0=ps, scalar1=bq_sb[:, m:m + 1])
                        else:
                            nc.vector.tensor_copy(out=dst[m][n], in_=ps)
                    return f

                def v_group(t):
                    def f():
                        tp, j = t // 2, t % 2
                        psv = pp.tile([P, NQ], f32, tag="pp_t", name="psv")
                        if t < NVB:
                            for a in range(ET):
                                nc.tensor.matmul(
                                    psv[:, :CW],
                                    lhsT=xtb_sb[:, a, t * P:(t + 1) * P],
                                    rhs=wv_sb[:, a, :],
                                    start=(a == 0), stop=(a == ET - 1),
                                )
                            pv = psv[:, :CW].rearrange("p (h d) -> p h d", h=HG)
                            nc.vector.tensor_copy(
                                out=vo_bf[t][:, 0::2, 0:D], in_=pv[:, 0::2, :])
                            nc.vector.tensor_copy(
                                out=vo_bf[t][:, 1::2, D:P], in_=pv[:, 1::2, :])
                            # gpsimd cannot read PSUM: cast the fp8 copy from
                            # the bf16 SBUF tile instead
                            nc.gpsimd.tensor_copy(
                                out=vop_t[tp][:, j, :, :],
                                in_=vo_bf[t][:, :, :])
                        else:
                            for i in range(AP2):
                                nc.tensor.matmul(
                                    psv[:, :CW],
                                    lhsT=xt_sb[:, 2 * i:2 * i + 2, t * P:(t + 1) * P],
                                    rhs=wv8_sb[:, 2 * i:2 * i + 2, :],
                                    start=(i == 0), stop=(i == AP2 - 1),
                                    perf_mode=DR,
                                )
                            pv = psv[:, :CW].rearrange("p (h d) -> p h d", h=HG)
                            nc.vector.tensor_copy(
                                out=vop_t[tp][:, j, 0::2, 0:D], in_=pv[:, 0::2, :])
                            nc.vector.tensor_copy(
                                out=vop_t[tp][:, j, 1::2, D:P], in_=pv[:, 1::2, :])
                    return f

                fs = []
                for m in range(MT):
                    fs.append(qk_group(wq_sb, qt_t, m, True))
                    fs.append(qk_group(wk_sb, kt_t, m, False))
                for t in range(4 * n, 4 * n + 4):
                    fs.append(v_group(t))
                return fs

            def wo_closures(qc):
                def wo_group(ti, ec):
                    def f():
                        wp = pp.tile([P, NQ], f32, tag="pp_t", name="wp")
                        for m in range(MT):
                            nc.tensor.matmul(
                                wp,
                                lhsT=ot_t[m][qc][:, (ti % 4) * P:(ti % 4 + 1) * P],
                                rhs=wo_sb[:, m, ec * NQ:(ec + 1) * NQ],
                                start=(m == 0), stop=(m == MT - 1),
                            )
                        so = ost.tile([P, NQ], f32, tag="ost", name="so")
                        if ec % 2 == 0:
                            nc.scalar.copy(out=so, in_=wp)
                        else:
                            nc.vector.tensor_copy(out=so, in_=wp)
                        nc.sync.dma_start(
                            out=out[ti * P:(ti + 1) * P, ec * NQ:(ec + 1) * NQ], in_=so)
                    return f
                return [wo_group(ti, ec)
                        for ti in range(4 * qc, 4 * qc + 4) for ec in range(E // NQ)]

            def wo_m0_closures(qc):
                # first-half contraction (heads 0/1), staged to SBUF f32;
                # runs hidden inside the final pair's attention
                def g(ti, ec):
                    def f():
                        wp = pp.tile([P, NQ], f32, tag="pp_t", name="wp0")
                        nc.tensor.matmul(
                            wp,
                            lhsT=ot_t[0][qc][:, (ti % 4) * P:(ti % 4 + 1) * P],
                            rhs=wo_sb[:, 0, ec * NQ:(ec + 1) * NQ],
                            start=True, stop=True)
                        s = wo_stage[(ti % 4) * 2 + ec]
                        if ec % 2 == 0:
                            nc.scalar.copy(out=s, in_=wp)
                        else:
                            nc.vector.tensor_copy(out=s, in_=wp)
                    return f
                return [g(ti, ec)
                        for ti in range(4 * qc, 4 * qc + 4) for ec in range(E // NQ)]

            def wo_m1_closures(qc):
                def g(ti, ec):
                    def f():
                        wp = pp.tile([P, NQ], f32, tag="pp_t", name="wp1")
                        nc.tensor.matmul(
                            wp,
                            lhsT=ot_t[1][qc][:, (ti % 4) * P:(ti % 4 + 1) * P],
                            rhs=wo_sb[:, 1, ec * NQ:(ec + 1) * NQ],
                            start=True, stop=True)
                        so = ost.tile([P, NQ], f32, tag="ost", name="so")
                        nc.vector.tensor_add(
                            so, wp, wo_stage[(ti % 4) * 2 + ec])
                        nc.sync.dma_start(
                            out=out[ti * P:(ti + 1) * P, ec * NQ:(ec + 1) * NQ], in_=so)
                    return f
                return [g(ti, ec)
                        for ti in range(4 * qc, 4 * qc + 4) for ec in range(E // NQ)]

            def pair_stream(qc, pair):
                nkt = 4 * qc + 4
                dr_all = PH2 and qc >= 1
                o_ps = {}

                def alloc():
                    for h in pair:
                        o_ps[h] = op.tile([P, NQ], f32, tag="o_ps", name=f"o_ps{h}")

                def ebatch(ktb, u_ts):
                    kts = (ktb, ktb + 1)
                    offs = [max(0, (kt - 4 * qc) * P) for kt in kts]
                    off0, off1 = offs
                    diag = ktb >= 4 * qc
                    e_ts = {}
                    for h in pair:
                        e_ts[h] = ep.tile([P, 2 * NQ], f32, tag="e_ps",
                                          name=f"e_ps{h}")
                    # alternate heads so adjacent matmuls use disjoint PE
                    # row groups (base partitions 0/64): the 64x128 row tiles
                    # run the two heads' streams concurrently
                    for j, kt in enumerate(kts):
                        eoff = offs[j]
                        for h in pair:
                            m, r0 = h // 2, 64 * (h % 2)
                            nc.tensor.matmul(
                                e_ts[h][:, j * NQ + eoff:(j + 1) * NQ],
                                lhsT=kt_t[m][kt // 4][r0:r0 + D,
                                                      (kt % 4) * P:(kt % 4 + 1) * P],
                                rhs=qt_t[m][qc][r0:r0 + D, eoff:NQ],
                                start=True, stop=True,
                            )
                    for h in pair:
                        if diag:
                            # additive causal mask on PSUM: j=0 diag block,
                            # then j=1 diag block
                            nc.vector.tensor_add(
                                e_ts[h][:, off0:off0 + P],
                                e_ts[h][:, off0:off0 + P], tn_sb)
                            nc.vector.tensor_add(
                                e_ts[h][:, NQ + off1:NQ + off1 + P],
                                e_ts[h][:, NQ + off1:NQ + off1 + P], tn_sb)
                            if dr_all:
                                # fp8 U consumed by a DR matmul: one exp over
                                # the whole span (the dead strip exps psum
                                # garbage), then zero the strip so the DR
                                # stream multiplies zeros there
                                ut = up.tile([P, 2 * NQ], f8, tag="u", name=f"u{h}")
                                u_ts[h] = ut
                                nc.scalar.activation(
                                    ut[:, off0:], e_ts[h][:, off0:],
                                    Exp, scale=float(SCALE))
                                nc.gpsimd.memset(ut[:, NQ + off0:NQ + off1], 0.0)
                                continue
                            ut = ub.tile([P, 2 * NQ], bf16, tag="ub", name=f"ub{h}")
                        else:
                            ut = up.tile([P, 2 * NQ], f8, tag="u", name=f"u{h}")
                        u_ts[h] = ut
                        nc.scalar.activation(
                            ut[:, off0:], e_ts[h][:, off0:],
                            Exp, scale=float(SCALE))
                    if debug_taps and qc == 0 and pair == (0, 1) and ktb == 0:
                        nc.sync.dma_start(out=dbg["u000"][:, :], in_=u_ts[0])

                def avbatch(ktb, u_ts):
                    offs = [max(0, (kt - 4 * qc) * P) for kt in (ktb, ktb + 1)]
                    off0 = offs[0]
                    diag = ktb >= 4 * qc
                    tp = ktb // 2
                    for h in pair:
                        if dr_all or not diag:
                            uv = u_ts[h].rearrange("p (j q) -> p j q", j=2)
                            stop_kt = (4 * qc - 2) if not dr_all else (nkt - 2)
                            nc.tensor.matmul(
                                o_ps[h][:, off0:NQ],
                                lhsT=vop_t[tp][:, :, h, :],
                                rhs=uv[:, :, off0:NQ],
                                start=(ktb == 0), stop=(ktb == stop_kt),
                                perf_mode=DR,
                                skip_group_check=True,
                            )
                        else:
                            for j, kt in enumerate((ktb, ktb + 1)):
                                off = offs[j]
                                nc.tensor.matmul(
                                    o_ps[h][:, off:NQ],
                                    lhsT=vo_bf[kt][:, h, :],
                                    rhs=u_ts[h][:, j * NQ + off:(j + 1) * NQ],
                                    start=(kt == 0),
                                    stop=(kt == nkt - 1),
                                    skip_group_check=True,
                                )

                def norm(h):
                    # hw partition_broadcast reads partition 0 regardless of
                    # the input AP offset (and the custom reciprocal is only
                    # reliable at partition 0). Even heads: V sums at PSUM
                    # rows 0:64, denominator at row 64, DMA-shifted to row 0.
                    # Odd heads: denominator already at row 0, V at 64:128,
                    # no shifts, and the mul writes ot rows 64:128 in-lane.
                    m = h // 2
                    dn = sm.tile([P, NQ], f32, tag="dn", name="dn")
                    rc = sm.tile([P, NQ], f32, tag="rc", name="rc")
                    bc = sm.tile([P, NQ], f32, tag="bc", name="bc")
                    if h % 2 == 0:
                        nc.vector.tensor_copy(out=dn[D:D + 1, :], in_=o_ps[h][D:D + 1, :])
                        nc.sync.dma_start(out=dn[0:1, :], in_=dn[D:D + 1, :])
                        nc.vector.reciprocal_approx_fast(out=rc[0:1, :], in_=dn[0:1, :])
                        nc.gpsimd.partition_broadcast(bc[0:D, :], rc[0:1, :], channels=D)
                        nc.vector.tensor_mul(
                            ot_t[m][qc][0:D, :], o_ps[h][0:D, :], bc[0:D, :])
                    else:
                        nc.vector.tensor_copy(out=dn[0:1, :], in_=o_ps[h][0:1, :])
                        nc.vector.reciprocal_approx_fast(out=rc[0:1, :], in_=dn[0:1, :])
                        # broadcast ignores the output partition offset, so
                        # fill all 128 partitions and use the top half
                        nc.gpsimd.partition_broadcast(bc[:, :], rc[0:1, :], channels=P)
                        nc.vector.tensor_mul(
                            ot_t[m][qc][D:P, :], o_ps[h][D:P, :], bc[D:P, :])

                alloc()
                for ktb in range(0, nkt, 2):
                    u_ts = {}
                    yield (lambda ktb=ktb, u_ts=u_ts: ebatch(ktb, u_ts))
                    yield (lambda ktb=ktb, u_ts=u_ts: avbatch(ktb, u_ts))
                for h in pair:
                    yield (lambda h=h: norm(h))

            def run_slots(slots, fillers):
                # distribute fillers across the attention slots only (the
                # last two slots are the norms): every filler's engine ops
                # must precede the norm chain, because the norm muls wait on
                # the slow gpsimd broadcast and any PE-feeding DVE op behind
                # them (the next chunk's qt/kt evac) would head-of-line
                # block on the in-order DVE queue
                nf, ns, fi = len(fillers), max(len(slots) - 2, 1), 0
                for i, sf in enumerate(slots):
                    if i >= ns:
                        while fi < nf:
                            fillers[fi]()
                            fi += 1
                    sf()
                    want = min((i + 1) * nf // ns, nf)
                    while fi < want:
                        fillers[fi]()
                        fi += 1

            # emit only what the first attention pair needs up front (heads
            # 0/1 projections + the first two V tiles); the rest interleaves
            # into the first pair's exp-gated gaps
            pc0 = proj_closures(0)
            for f in (pc0[0], pc0[1], pc0[4], pc0[5]):
                f()
            pre = [pc0[6], pc0[7], pc0[2], pc0[3]]
            for qc in range(QC):
                fillers = []
                if qc == 0:
                    fillers += pre
                if qc + 1 < QC:
                    fillers += proj_closures(qc + 1)
                if qc >= 1:
                    fillers += wo_closures(qc - 1)
                s1 = list(pair_stream(qc, (0, 1)))
                s2 = list(pair_stream(qc, (2, 3)))
                half = len(fillers) // 2
                f1, f2 = fillers[:half], fillers[half:]
                if qc == QC - 1:
                    # heads 0/1 are normalized by the end of s1, so the m=0
                    # Wo half hides inside the final pair's attention
                    f2 = f2 + wo_m0_closures(qc)
                run_slots(s1, f1)
                run_slots(s2, f2)
            for f in wo_m1_closures(QC - 1):
                f()
            if debug_taps:
                nc.sync.dma_start(out=dbg["qt00"][:, :], in_=qt_t[0][0])
                nc.sync.dma_start(out=dbg["kt00"][:, :], in_=kt_t[0][0])
                nc.sync.dma_start(out=dbg["vo0"][:, :],
                                  in_=vo_bf[0].rearrange("p h d -> p (h d)"))
                nc.sync.dma_start(out=dbg["ot00"][:, :], in_=ot_t[0][0])
    nc.compile()
    return nc


def _prepare_in_maps(x, Wq, bq, Wk, Wv, Wo):
    import ml_dtypes
    bfd = ml_dtypes.bfloat16
    f8d = ml_dtypes.float8_e4m3fn
    tn = np.tril(np.full((P, P), NEG, np.float32), -1)
    xl = [np.ascontiguousarray(
        x[b].T.reshape(ET, P, T).transpose(1, 0, 2)) for b in range(B)]
    xtb8 = [a.astype(f8d) for a in xl]
    if PH2:
        xtbb = [np.ascontiguousarray(a[:, :, :NQ]).astype(bfd) for a in xl]
    else:
        xtbb = [a.astype(bfd) for a in xl]
    in_maps = []
    for c in range(NCORES):
        b, g = c // G, c % G
        cs = slice(g * CW, (g + 1) * CW)
        bq_g = np.ascontiguousarray(bq[cs].reshape(MT, P).T)
        def wlay(w, dt):  # [E, CW] -> [P, ET, CW] with e = a*P + p
            return np.ascontiguousarray(
                w.reshape(ET, P, CW).transpose(1, 0, 2)).astype(dt)
        wo_l = np.ascontiguousarray(
            Wo[cs, :].reshape(MT, P, E).transpose(1, 0, 2)).astype(bfd)
        in_maps.append({
            "xt": xtb8[b],
            "xtb": xtbb[b],
            "wq": wlay(Wq[:, cs], f8d),
            "wk": wlay(Wk[:, cs], f8d),
            "wv": wlay(Wv[:, cs], bfd),
            "wv8": wlay(Wv[:, cs], f8d),
            "bq": bq_g,
            "wo": wo_l,
            "tn": tn,
        })
    return in_maps


def _run(inputs, trace=False, trace_kwargs=None, debug_taps=False):
    from concourse.bass_utils import run_bass_kernel_spmd

    key = ("nc", debug_taps)
    if key not in _CACHE:
        _CACHE[key] = _build_bass(debug_taps=debug_taps)
    nc = _CACHE[key]

    x = np.asarray(inputs["x"], np.float32)
    Wq = np.asarray(inputs["Wq"], np.float32)
    Wk = np.asarray(inputs["Wk"], np.float32)
    Wv = np.asarray(inputs["Wv"], np.float32)
    Wo = np.asarray(inputs["Wo"], np.float32)
    bq = np.asarray(inputs["bq"], np.float32)
    bv = np.asarray(inputs["bv"], np.float32)
    bo = np.asarray(inputs["bo"], np.float32)

    in_maps = _prepare_in_maps(x, Wq, bq, Wk, Wv, Wo)
    res = run_bass_kernel_spmd(
        nc, in_maps, core_ids=list(range(NCORES)),
        trace=trace, **(trace_kwargs or {}))

    bias_row = (bv @ Wo + bo).astype(np.float32)
    y = np.empty((B, T, E), np.float32)
    for b in range(B):
        acc = res.results[G * b]["out"].astype(np.float32).copy()
        for g in range(1, G):
            acc += res.results[G * b + g]["out"]
        y[b] = acc + bias_row
    return y, res


def kernel(**inputs) -> np.ndarray:
    return _run(inputs, trace=False)[0]


# revision 25
# speedup vs baseline: 1.1516x; 1.1516x over previous
"""Causal multi-head attention layer for Trainium2, sharded over 8 NeuronCores.

Problem: B=2, T=2048, E=1024, H=16 heads (D=64), fp32.
  out = softmax(mask(QK^T)/sqrt(E)) V Wo + bo   with Q=xWq+bq etc.

Sharding: data-parallel over batch (2) x tensor-parallel over head groups (4):
core c -> batch b=c//4, head group g=c%4 (4 heads, 256 channels).
Each core computes partial = attn_heads(x_b) @ Wo[rows of g]; host sums the 4
partials per batch and adds the bias row.

Math folds used (all exact):
 - bk drops out of softmax (additive shift along the softmax axis).
 - attn rows sum to 1  =>  attn @ (V + 1 bv^T) = attn@V + 1 bv^T, so bv enters
   the output as the constant row bv @ Wo, added on the host with bo.
 - bq is added to Q^T on-chip (per-partition bias).

Measured PE law (trace): effective clock duty-cycles ~1.2-2.4GHz (power
throttle), bf16/fp8 stream 1 col/cycle, fp8e4 DoubleRow streams 2 phys cols
per cycle for a 256-deep contraction. So DR fp8 ~2x bf16 throughput; plain
fp8 = bf16 speed.

Precision: fp8 error concentrates in rows whose softmax averages few keys
(early rows). Chunk 0 (q<512) keeps a bf16 V + bf16 diag path; chunks 1-3
run the causal-diagonal kt-pairs in fp8 DR like the off-diagonal (those rows
attend >=512 keys). V is projected in fp8 DR for key tiles 4..15 and bf16 for
tiles 0..3.

Device layout per core (matmuls: out = lhsT.T @ rhs, contraction on
partitions):
 - Q^T/K^T = W^T x^T in fp8 DoubleRow over e-tile pairs -> bf16 [ch, t].
 - V tiles 0..3: bf16 x W -> vo_bf [k, h, D+1] (ones col for the denominator)
   + fp8 cast into vop; tiles 4..15: fp8 DR -> vop [k, ktpair, h, 128-pad]
   directly (ones row at 64, zero pad above).
 - energy^T = K Q^T per head, bf16, d=64 contraction at PE row base 0/64
   (64x128 row tiling runs the two heads of a pair concurrently); causal
   mask added (-1e9) on PSUM before exp.
 - exp on ACT: chunk 0 diag pairs -> bf16 U; everything else -> fp8 U (diag
   pairs split j0/j1 + gpsimd memset of the masked gap so DR sees zeros).
 - O^T accumulates per head in PSUM: one fp8 DR matmul per kt-pair (V rows
   0..63, ones denominator row 64, zero pad above); chunk 0 diag in bf16
   M=65 matmuls.
 - normalize with DVE mult by gpsimd-broadcast reciprocal; the final pair
   uses a PE outer-product broadcast (fp32r) instead so the tail chain is
   short; odd heads shift to partitions 64..127 via SBUF->SBUF DMA.
 - partial = (O^T).T @ Wo rows in bf16, PSUM -> SBUF -> DRAM. The last
   chunk's Wo is split: m=0 half staged to SBUF during the final pair's
   attention, m=1 half + DVE add + DMA after the last norm.
"""

import os
import numpy as np

B, T, E, H = 2, 2048, 1024, 16
P = 128
NCORES = 8
G = 4            # head groups (tensor parallel)
HG = H // G      # heads per group = 4
D = E // H       # 64
CW = HG * D      # channels per group = 256
ET = E // P      # 8 e-tiles
AP2 = ET // 2    # 4 e-tile pairs
MT = CW // P     # 2 hd-tiles
TT = T // P      # 16 t-tiles
NQ = 512         # q-chunk width
QC = T // NQ     # 4 q-chunks
NEG = -1.0e9
SCALE = 1.0 / np.sqrt(E).astype(np.float32)  # 1/32

PH2 = True           # fp8-DR V (tiles>=4) + fp8-DR diag AV (qc>=1)

_CACHE: dict = {}


def _build_bass(debug_taps=False):
    import concourse.bass as bass
    import concourse.mybir as mybir
    import concourse.tile as tile
    from concourse import bacc

    f32 = mybir.dt.float32
    bf16 = mybir.dt.bfloat16
    f8 = mybir.dt.float8e4
    DR = mybir.MatmulPerfMode.DoubleRow
    Exp = mybir.ActivationFunctionType.Exp

    nc = bacc.Bacc("TRN2", target_bir_lowering=False, name="attn_tp")
    dbg = {}
    if debug_taps:
        dbg["qt00"] = nc.dram_tensor("dbg_qt00", [P, NQ], bf16, kind="ExternalOutput")
        dbg["kt00"] = nc.dram_tensor("dbg_kt00", [P, NQ], bf16, kind="ExternalOutput")
        dbg["vo0"] = nc.dram_tensor("dbg_vo0", [P, HG * (D + 1)], bf16, kind="ExternalOutput")
        dbg["u000"] = nc.dram_tensor("dbg_u000", [P, 2 * NQ], bf16, kind="ExternalOutput")
        dbg["ot00"] = nc.dram_tensor("dbg_ot00", [P, NQ], bf16, kind="ExternalOutput")
    XTBW = NQ if PH2 else T
    xt = nc.dram_tensor("xt", [P, ET, T], f8, kind="ExternalInput")
    xtb = nc.dram_tensor("xtb", [P, ET, XTBW], bf16, kind="ExternalInput")
    wq = nc.dram_tensor("wq", [P, ET, CW], f8, kind="ExternalInput")
    wk = nc.dram_tensor("wk", [P, ET, CW], f8, kind="ExternalInput")
    wv = nc.dram_tensor("wv", [P, ET, CW], bf16, kind="ExternalInput")
    wv8 = nc.dram_tensor("wv8", [P, ET, CW], f8, kind="ExternalInput")
    bq = nc.dram_tensor("bq", [P, MT], f32, kind="ExternalInput")
    wo = nc.dram_tensor("wo", [P, MT, E], bf16, kind="ExternalInput")
    tn = nc.dram_tensor("tn", [P, P], f32, kind="ExternalInput")
    out = nc.dram_tensor("out", [T, E], f32, kind="ExternalOutput")

    NVB = 4 if PH2 else TT   # number of bf16 V tiles

    with tile.TileContext(nc) as tc:
        with (
            tc.tile_pool(name="persist", bufs=1) as pers,
            tc.tile_pool(name="pp", bufs=2, space="PSUM") as pp,
            tc.tile_pool(name="ep", bufs=2, space="PSUM") as ep,
            tc.tile_pool(name="op", bufs=2, space="PSUM") as op,
            tc.tile_pool(name="up", bufs=6) as up,
            tc.tile_pool(name="ub", bufs=4) as ub,
            tc.tile_pool(name="sm", bufs=4) as sm,
            tc.tile_pool(name="ost", bufs=6) as ost,
        ):
            # ---- persistent SBUF tensors ----
            xt_sb = pers.tile([P, ET, T], f8, tag="xt_sb", name="xt_sb")
            xtb_sb = pers.tile([P, ET, XTBW], bf16, tag="xtb_sb", name="xtb_sb")
            wq_sb = pers.tile([P, ET, CW], f8, tag="wq_sb", name="wq_sb")
            wk_sb = pers.tile([P, ET, CW], f8, tag="wk_sb", name="wk_sb")
            wv_sb = pers.tile([P, ET, CW], bf16, tag="wv_sb", name="wv_sb")
            wv8_sb = pers.tile([P, ET, CW], f8, tag="wv8_sb", name="wv8_sb")
            wo_sb = pers.tile([P, MT, E], bf16, tag="wo_sb", name="wo_sb")
            bq_sb = pers.tile([P, MT], f32, tag="bq_sb", name="bq_sb")
            tn_sb = pers.tile([P, P], f32, tag="tn_sb", name="tn_sb")
            qt_t = [[pers.tile([P, NQ], bf16, tag=f"qt{m}_{n}", name=f"qt{m}_{n}")
                     for n in range(QC)] for m in range(MT)]
            kt_t = [[pers.tile([P, NQ], bf16, tag=f"kt{m}_{n}", name=f"kt{m}_{n}")
                     for n in range(QC)] for m in range(MT)]
            vo_bf = [pers.tile([P, HG, P], bf16, tag=f"vb{t}", name=f"vb{t}")
                     for t in range(NVB)]
            vop_t = [pers.tile([P, 2, HG, P], f8, tag=f"vo{tp}", name=f"vo{tp}")
                     for tp in range(TT // 2)]
            ot_t = [[pers.tile([P, NQ], bf16, tag=f"ot{m}_{n}", name=f"ot{m}_{n}")
                     for n in range(QC)] for m in range(MT)]
            wo_stage = [pers.tile([P, NQ], f32, tag=f"ws{g}", name=f"ws{g}")
                        for g in range(8)]

            # stationary V layout per (kt, head): parity-split so the AV
            # output lands where the ot tile wants it with no partition
            # shift: even heads [V 0:64 | ones@64 | zeros], odd heads
            # [ones@0 | zeros | V 64:128]. The ones row makes the DR matmul
            # emit the softmax denominator (even: PSUM row 64, odd: row 0).
            # Static ones/zero regions are initialized once on gpsimd.
            for tp in range(TT // 2):
                nc.gpsimd.memset(vop_t[tp][:, :, 0::2, D:D + 1], 1.0)
                nc.gpsimd.memset(vop_t[tp][:, :, 0::2, D + 1:], 0.0)
                nc.gpsimd.memset(vop_t[tp][:, :, 1::2, 0:1], 1.0)
                nc.gpsimd.memset(vop_t[tp][:, :, 1::2, 1:D], 0.0)
            for t in range(NVB):
                nc.gpsimd.memset(vo_bf[t][:, 0::2, D:D + 1], 1.0)
                nc.gpsimd.memset(vo_bf[t][:, 0::2, D + 1:], 0.0)
                nc.gpsimd.memset(vo_bf[t][:, 1::2, 0:1], 1.0)
                nc.gpsimd.memset(vo_bf[t][:, 1::2, 1:D], 0.0)

            # ---- input DMAs ----
            # weights first so the first projection starts early; x arrives
            # in column chunks so chunk-n projections do not wait on full x
            nc.scalar.dma_start(out=wq_sb, in_=wq[:, :, :])
            nc.scalar.dma_start(out=wk_sb, in_=wk[:, :, :])
            nc.scalar.dma_start(out=bq_sb, in_=bq[:, :])
            # chunk-0 columns split by e-tile pair so the first projection
            # matmul starts after the first quarter arrives
            for i in range(AP2):
                nc.sync.dma_start(out=xt_sb[:, 2 * i:2 * i + 2, 0:NQ],
                                  in_=xt[:, 2 * i:2 * i + 2, 0:NQ])
            nc.scalar.dma_start(out=wv_sb, in_=wv[:, :, :])
            nc.scalar.dma_start(out=wv8_sb, in_=wv8[:, :, :])
            nc.sync.dma_start(out=xtb_sb[:, :, 0:XTBW], in_=xtb[:, :, 0:XTBW])
            for n in range(1, QC):
                nc.sync.dma_start(out=xt_sb[:, :, n * NQ:(n + 1) * NQ],
                                  in_=xt[:, :, n * NQ:(n + 1) * NQ])
                if not PH2 and n < QC:
                    nc.scalar.dma_start(out=xtb_sb[:, :, n * NQ:(n + 1) * NQ],
                                        in_=xtb[:, :, n * NQ:(n + 1) * NQ])
            nc.scalar.dma_start(out=tn_sb, in_=tn[:, :])
            nc.scalar.dma_start(out=wo_sb, in_=wo[:, :, :])

            # ---- software-pipelined emission ----
            # PE engine queues are in-order, so attention batches (gated on
            # ACT exp) are interleaved with independent filler work: the next
            # chunk's projection groups and the previous chunk's Wo groups.

            def proj_closures(n):
                def qk_group(wsb, dst, m, biased):
                    def f():
                        ps = pp.tile([P, NQ], f32, tag="pp_t", name="psqk")
                        for i in range(AP2):
                            nc.tensor.matmul(
                                ps,
                                lhsT=wsb[:, 2 * i:2 * i + 2, m * P:(m + 1) * P],
                                rhs=xt_sb[:, 2 * i:2 * i + 2,
                                          n * NQ:(n + 1) * NQ],
                                start=(i == 0), stop=(i == AP2 - 1),
                                perf_mode=DR,
                            )
                        if biased:
                            nc.vector.tensor_scalar_add(
                                out=dst[m][n], in0=ps, scalar1=bq_sb[:, m:m + 1])
                        else:
                            nc.vector.tensor_copy(out=dst[m][n], in_=ps)
                    return f

                def v_group(t):
                    def f():
                        tp, j = t // 2, t % 2
                        psv = pp.tile([P, NQ], f32, tag="pp_t", name="psv")
                        if t < NVB:
                            for a in range(ET):
                                nc.tensor.matmul(
                                    psv[:, :CW],
                                    lhsT=xtb_sb[:, a, t * P:(t + 1) * P],
                                    rhs=wv_sb[:, a, :],
                                    start=(a == 0), stop=(a == ET - 1),
                                )
                            pv = psv[:, :CW].rearrange("p (h d) -> p h d", h=HG)
                            nc.vector.tensor_copy(
                                out=vo_bf[t][:, 0::2, 0:D], in_=pv[:, 0::2, :])
                            nc.vector.tensor_copy(
                                out=vo_bf[t][:, 1::2, D:P], in_=pv[:, 1::2, :])
                            # gpsimd cannot read PSUM: cast the fp8 copy from
                            # the bf16 SBUF tile instead
                            nc.gpsimd.tensor_copy(
                                out=vop_t[tp][:, j, :, :],
                                in_=vo_bf[t][:, :, :])
                        else:
                            for i in range(AP2):
                                nc.tensor.matmul(
                                    psv[:, :CW],
                                    lhsT=xt_sb[:, 2 * i:2 * i + 2, t * P:(t + 1) * P],
                                    rhs=wv8_sb[:, 2 * i:2 * i + 2, :],
                                    start=(i == 0), stop=(i == AP2 - 1),
                                    perf_mode=DR,
                                )
                            pv = psv[:, :CW].rearrange("p (h d) -> p h d", h=HG)
                            nc.vector.tensor_copy(
                                out=vop_t[tp][:, j, 0::2, 0:D], in_=pv[:, 0::2, :])
                            nc.vector.tensor_copy(
                                out=vop_t[tp][:, j, 1::2, D:P], in_=pv[:, 1::2, :])
                    return f

                fs = []
                for m in range(MT):
                    fs.append(qk_group(wq_sb, qt_t, m, True))
                    fs.append(qk_group(wk_sb, kt_t, m, False))
                for t in range(4 * n, 4 * n + 4):
                    fs.append(v_group(t))
                return fs

            def wo_closures(qc):
                def wo_group(ti, ec):
                    def f():
                        wp = pp.tile([P, NQ], f32, tag="pp_t", name="wp")
                        for m in range(MT):
                            nc.tensor.matmul(
                                wp,
                                lhsT=ot_t[m][qc][:, (ti % 4) * P:(ti % 4 + 1) * P],
                                rhs=wo_sb[:, m, ec * NQ:(ec + 1) * NQ],
                                start=(m == 0), stop=(m == MT - 1),
                            )
                        so = ost.tile([P, NQ], f32, tag="ost", name="so")
                        if ec % 2 == 0:
                            nc.scalar.copy(out=so, in_=wp)
                        else:
                            nc.vector.tensor_copy(out=so, in_=wp)
                        nc.sync.dma_start(
                            out=out[ti * P:(ti + 1) * P, ec * NQ:(ec + 1) * NQ], in_=so)
                    return f
                return [wo_group(ti, ec)
                        for ti in range(4 * qc, 4 * qc + 4) for ec in range(E // NQ)]

            def wo_m0_closures(qc):
                # first-half contraction (heads 0/1), staged to SBUF f32;
                # runs hidden inside the final pair's attention
                def g(ti, ec):
                    def f():
                        wp = pp.tile([P, NQ], f32, tag="pp_t", name="wp0")
                        nc.tensor.matmul(
                            wp,
                            lhsT=ot_t[0][qc][:, (ti % 4) * P:(ti % 4 + 1) * P],
                            rhs=wo_sb[:, 0, ec * NQ:(ec + 1) * NQ],
                            start=True, stop=True)
                        s = wo_stage[(ti % 4) * 2 + ec]
                        if ec % 2 == 0:
                            nc.scalar.copy(out=s, in_=wp)
                        else:
                            nc.vector.tensor_copy(out=s, in_=wp)
                    return f
                return [g(ti, ec)
                        for ti in range(4 * qc, 4 * qc + 4) for ec in range(E // NQ)]

            def wo_m1_closures(qc):
                def g(ti, ec):
                    def f():
                        wp = pp.tile([P, NQ], f32, tag="pp_t", name="wp1")
                        nc.tensor.matmul(
                            wp,
                            lhsT=ot_t[1][qc][:, (ti % 4) * P:(ti % 4 + 1) * P],
                            rhs=wo_sb[:, 1, ec * NQ:(ec + 1) * NQ],
                            start=True, stop=True)
                        so = ost.tile([P, NQ], f32, tag="ost", name="so")
                        nc.vector.tensor_add(
                            so, wp, wo_stage[(ti % 4) * 2 + ec])
                        nc.sync.dma_start(
                            out=out[ti * P:(ti + 1) * P, ec * NQ:(ec + 1) * NQ], in_=so)
                    return f
                return [g(ti, ec)
                        for ti in range(4 * qc, 4 * qc + 4) for ec in range(E // NQ)]

            def pair_stream(qc, pair):
                nkt = 4 * qc + 4
                dr_all = PH2 and qc >= 1
                o_ps = {}

                def alloc():
                    for h in pair:
                        o_ps[h] = op.tile([P, NQ], f32, tag="o_ps", name=f"o_ps{h}")

                def ebatch(ktb, u_ts):
                    kts = (ktb, ktb + 1)
                    offs = [max(0, (kt - 4 * qc) * P) for kt in kts]
                    off0, off1 = offs
                    diag = ktb >= 4 * qc
                    e_ts = {}
                    for h in pair:
                        e_ts[h] = ep.tile([P, 2 * NQ], f32, tag="e_ps",
                                          name=f"e_ps{h}")
                    # alternate heads so adjacent matmuls use disjoint PE
                    # row groups (base partitions 0/64): the 64x128 row tiles
                    # run the two heads' streams concurrently
                    for j, kt in enumerate(kts):
                        eoff = offs[j]
                        for h in pair:
                            m, r0 = h // 2, 64 * (h % 2)
                            nc.tensor.matmul(
                                e_ts[h][:, j * NQ + eoff:(j + 1) * NQ],
                                lhsT=kt_t[m][kt // 4][r0:r0 + D,
                                                      (kt % 4) * P:(kt % 4 + 1) * P],
                                rhs=qt_t[m][qc][r0:r0 + D, eoff:NQ],
                                start=True, stop=True,
                            )
                    for h in pair:
                        if diag:
                            # additive causal mask on PSUM: j=0 diag block,
                            # then j=1 diag block
                            nc.vector.tensor_add(
                                e_ts[h][:, off0:off0 + P],
                                e_ts[h][:, off0:off0 + P], tn_sb)
                            nc.vector.tensor_add(
                                e_ts[h][:, NQ + off1:NQ + off1 + P],
                                e_ts[h][:, NQ + off1:NQ + off1 + P], tn_sb)
                            if dr_all:
                                # fp8 U consumed by a DR matmul: one exp over
                                # the whole span (the dead strip exps psum
                                # garbage), then zero the strip so the DR
                                # stream multiplies zeros there
                                ut = up.tile([P, 2 * NQ], f8, tag="u", name=f"u{h}")
                                u_ts[h] = ut
                                nc.scalar.activation(
                                    ut[:, off0:], e_ts[h][:, off0:],
                                    Exp, scale=float(SCALE))
                                nc.gpsimd.memset(ut[:, NQ + off0:NQ + off1], 0.0)
                                continue
                            ut = ub.tile([P, 2 * NQ], bf16, tag="ub", name=f"ub{h}")
                        else:
                            ut = up.tile([P, 2 * NQ], f8, tag="u", name=f"u{h}")
                        u_ts[h] = ut
                        nc.scalar.activation(
                            ut[:, off0:], e_ts[h][:, off0:],
                            Exp, scale=float(SCALE))
                    if debug_taps and qc == 0 and pair == (0, 1) and ktb == 0:
                        nc.sync.dma_start(out=dbg["u000"][:, :], in_=u_ts[0])

                def avbatch(ktb, u_ts):
                    offs = [max(0, (kt - 4 * qc) * P) for kt in (ktb, ktb + 1)]
                    off0 = offs[0]
                    diag = ktb >= 4 * qc
                    tp = ktb // 2
                    for h in pair:
                        if dr_all or not diag:
                            uv = u_ts[h].rearrange("p (j q) -> p j q", j=2)
                            stop_kt = (4 * qc - 2) if not dr_all else (nkt - 2)
                            nc.tensor.matmul(
                                o_ps[h][:, off0:NQ],
                                lhsT=vop_t[tp][:, :, h, :],
                                rhs=uv[:, :, off0:NQ],
                                start=(ktb == 0), stop=(ktb == stop_kt),
                                perf_mode=DR,
                                skip_group_check=True,
                            )
                        else:
                            for j, kt in enumerate((ktb, ktb + 1)):
                                off = offs[j]
                                nc.tensor.matmul(
                                    o_ps[h][:, off:NQ],
                                    lhsT=vo_bf[kt][:, h, :],
                                    rhs=u_ts[h][:, j * NQ + off:(j + 1) * NQ],
                                    start=(kt == 0),
                                    stop=(kt == nkt - 1),
                                    skip_group_check=True,
                                )

                def norm(h):
                    # hw partition_broadcast reads partition 0 regardless of
                    # the input AP offset (and the custom reciprocal is only
                    # reliable at partition 0). Even heads: V sums at PSUM
                    # rows 0:64, denominator at row 64, DMA-shifted to row 0.
                    # Odd heads: denominator already at row 0, V at 64:128,
                    # no shifts, and the mul writes ot rows 64:128 in-lane.
                    m = h // 2
                    dn = sm.tile([P, NQ], f32, tag="dn", name="dn")
                    rc = sm.tile([P, NQ], f32, tag="rc", name="rc")
                    bc = sm.tile([P, NQ], f32, tag="bc", name="bc")
                    if h % 2 == 0:
                        nc.vector.tensor_copy(out=dn[D:D + 1, :], in_=o_ps[h][D:D + 1, :])
                        nc.sync.dma_start(out=dn[0:1, :], in_=dn[D:D + 1, :])
                        nc.vector.reciprocal_approx_fast(out=rc[0:1, :], in_=dn[0:1, :])
                        nc.gpsimd.partition_broadcast(bc[0:D, :], rc[0:1, :], channels=D)
                        nc.vector.tensor_mul(
                            ot_t[m][qc][0:D, :], o_ps[h][0:D, :], bc[0:D, :])
                    else:
                        nc.vector.tensor_copy(out=dn[0:1, :], in_=o_ps[h][0:1, :])
                        nc.vector.reciprocal_approx_fast(out=rc[0:1, :], in_=dn[0:1, :])
                        # broadcast ignores the output partition offset, so
                        # fill all 128 partitions and use the top half
                        nc.gpsimd.partition_broadcast(bc[:, :], rc[0:1, :], channels=P)
                        nc.vector.tensor_mul(
                            ot_t[m][qc][D:P, :], o_ps[h][D:P, :], bc[D:P, :])

                alloc()
                for ktb in range(0, nkt, 2):
                    u_ts = {}
                    yield (lambda ktb=ktb, u_ts=u_ts: ebatch(ktb, u_ts))
                    yield (lambda ktb=ktb, u_ts=u_ts: avbatch(ktb, u_ts))
                for h in pair:
                    yield (lambda h=h: norm(h))

            def run_slots(slots, fillers):
                # distribute fillers across the attention slots only (the
                # last two slots are the norms): every filler's engine ops
                # must precede the norm chain, because the norm muls wait on
                # the slow gpsimd broadcast and any PE-feeding DVE op behind
                # them (the next chunk's qt/kt evac) would head-of-line
                # block on the in-order DVE queue
                nf, ns, fi = len(fillers), max(len(slots) - 2, 1), 0
                for i, sf in enumerate(slots):
                    if i >= ns:
                        while fi < nf:
                            fillers[fi]()
                            fi += 1
                    sf()
                    want = min((i + 1) * nf // ns, nf)
                    while fi < want:
                        fillers[fi]()
                        fi += 1

            # emit only what the first attention pair needs up front (heads
            # 0/1 projections + the first two V tiles); the rest interleaves
            # into the first pair's exp-gated gaps
            pc0 = proj_closures(0)
            for f in (pc0[0], pc0[1], pc0[4], pc0[5]):
                f()
            pre = [pc0[6], pc0[7], pc0[2], pc0[3]]
            for qc in range(QC):
                fillers = []
                if qc == 0:
                    fillers += pre
                if qc + 1 < QC:
                    fillers += proj_closures(qc + 1)
                if qc >= 1:
                    fillers += wo_closures(qc - 1)
                s1 = list(pair_stream(qc, (0, 1)))
                s2 = list(pair_stream(qc, (2, 3)))
                half = len(fillers) // 2
                f1, f2 = fillers[:half], fillers[half:]
                if qc == QC - 1:
                    # heads 0/1 are normalized by the end of s1, so the m=0
                    # Wo half hides inside the final pair's attention
                    f2 = f2 + wo_m0_closures(qc)
                run_slots(s1, f1)
                run_slots(s2, f2)
            for f in wo_m1_closures(QC - 1):
                f()
            if debug_taps:
                nc.sync.dma_start(out=dbg["qt00"][:, :], in_=qt_t[0][0])
                nc.sync.dma_start(out=dbg["kt00"][:, :], in_=kt_t[0][0])
                nc.sync.dma_start(out=dbg["vo0"][:, :],
                                  in_=vo_bf[0].rearrange("p h d -> p (h d)"))
                nc.sync.dma_start(out=dbg["ot00"][:, :], in_=ot_t[0][0])
    nc.compile()
    return nc


def _prepare_in_maps(x, Wq, bq, Wk, Wv, Wo):
    import ml_dtypes
    bfd = ml_dtypes.bfloat16
    f8d = ml_dtypes.float8_e4m3fn
    tn = np.tril(np.full((P, P), NEG, np.float32), -1)
    xl = [np.ascontiguousarray(
        x[b].T.reshape(ET, P, T).transpose(1, 0, 2)) for b in range(B)]
    xtb8 = [a.astype(f8d) for a in xl]
    if PH2:
        xtbb = [np.ascontiguousarray(a[:, :, :NQ]).astype(bfd) for a in xl]
    else:
        xtbb = [a.astype(bfd) for a in xl]
    in_maps = []
    for c in range(NCORES):
        b, g = c // G, c % G
        cs = slice(g * CW, (g + 1) * CW)
        bq_g = np.ascontiguousarray(bq[cs].reshape(MT, P).T)
        def wlay(w, dt):  # [E, CW] -> [P, ET, CW] with e = a*P + p
            return np.ascontiguousarray(
                w.reshape(ET, P, CW).transpose(1, 0, 2)).astype(dt)
        wo_l = np.ascontiguousarray(
            Wo[cs, :].reshape(MT, P, E).transpose(1, 0, 2)).astype(bfd)
        in_maps.append({
            "xt": xtb8[b],
            "xtb": xtbb[b],
            "wq": wlay(Wq[:, cs], f8d),
            "wk": wlay(Wk[:, cs], f8d),
            "wv": wlay(Wv[:, cs], bfd),
            "wv8": wlay(Wv[:, cs], f8d),
            "bq": bq_g,
            "wo": wo_l,
            "tn": tn,
        })
    return in_maps


def _run(inputs, trace=False, trace_kwargs=None, debug_taps=False):
    from concourse.bass_utils import run_bass_kernel_spmd

    key = ("nc", debug_taps)
    if key not in _CACHE:
        _CACHE[key] = _build_bass(debug_taps=debug_taps)
    nc = _CACHE[key]

    x = np.asarray(inputs["x"], np.float32)
    Wq = np.asarray(inputs["Wq"], np.float32)
    Wk = np.asarray(inputs["Wk"], np.float32)
    Wv = np.asarray(inputs["Wv"], np.float32)
    Wo = np.asarray(inputs["Wo"], np.float32)
    bq = np.asarray(inputs["bq"], np.float32)
    bv = np.asarray(inputs["bv"], np.float32)
    bo = np.asarray(inputs["bo"], np.float32)

    in_maps = _prepare_in_maps(x, Wq, bq, Wk, Wv, Wo)
    res = run_bass_kernel_spmd(
        nc, in_maps, core_ids=list(range(NCORES)),
        trace=trace, **(trace_kwargs or {}))

    bias_row = (bv @ Wo + bo).astype(np.float32)
    y = np.empty((B, T, E), np.float32)
    for b in range(B):
        acc = res.results[G * b]["out"].astype(np.float32).copy()
        for g in range(1, G):
            acc += res.results[G * b + g]["out"]
        y[b] = acc + bias_row
    return y, res


def kernel(**inputs) -> np.ndarray:
    return _run(inputs, trace=False)[0]


# revision 26
# speedup vs baseline: 1.1532x; 1.0014x over previous
"""Causal multi-head attention layer for Trainium2, sharded over 8 NeuronCores.

Problem: B=2, T=2048, E=1024, H=16 heads (D=64), fp32.
  out = softmax(mask(QK^T)/sqrt(E)) V Wo + bo   with Q=xWq+bq etc.

Sharding: data-parallel over batch (2) x tensor-parallel over head groups (4):
core c -> batch b=c//4, head group g=c%4 (4 heads, 256 channels).
Each core computes partial = attn_heads(x_b) @ Wo[rows of g]; host sums the 4
partials per batch and adds the bias row.

Math folds used (all exact):
 - bk drops out of softmax (additive shift along the softmax axis).
 - attn rows sum to 1  =>  attn @ (V + 1 bv^T) = attn@V + 1 bv^T, so bv enters
   the output as the constant row bv @ Wo, added on the host with bo.
 - bq is added to Q^T on-chip (per-partition bias).

Measured PE law (trace): effective clock duty-cycles ~1.2-2.4GHz (power
throttle), bf16/fp8 stream 1 col/cycle, fp8e4 DoubleRow streams 2 phys cols
per cycle for a 256-deep contraction. So DR fp8 ~2x bf16 throughput; plain
fp8 = bf16 speed.

Precision: fp8 error concentrates in rows whose softmax averages few keys
(early rows). Chunk 0 (q<512) keeps a bf16 V + bf16 diag path; chunks 1-3
run the causal-diagonal kt-pairs in fp8 DR like the off-diagonal (those rows
attend >=512 keys). V is projected in fp8 DR for key tiles 4..15 and bf16 for
tiles 0..3.

Device layout per core (matmuls: out = lhsT.T @ rhs, contraction on
partitions):
 - Q^T/K^T = W^T x^T in fp8 DoubleRow over e-tile pairs -> bf16 [ch, t].
 - V tiles 0..3: bf16 x W -> vo_bf [k, h, D+1] (ones col for the denominator)
   + fp8 cast into vop; tiles 4..15: fp8 DR -> vop [k, ktpair, h, 128-pad]
   directly (ones row at 64, zero pad above).
 - energy^T = K Q^T per head, bf16, d=64 contraction at PE row base 0/64
   (64x128 row tiling runs the two heads of a pair concurrently); causal
   mask added (-1e9) on PSUM before exp.
 - exp on ACT: chunk 0 diag pairs -> bf16 U; everything else -> fp8 U (diag
   pairs split j0/j1 + gpsimd memset of the masked gap so DR sees zeros).
 - O^T accumulates per head in PSUM: one fp8 DR matmul per kt-pair (V rows
   0..63, ones denominator row 64, zero pad above); chunk 0 diag in bf16
   M=65 matmuls.
 - normalize with DVE mult by gpsimd-broadcast reciprocal; the final pair
   uses a PE outer-product broadcast (fp32r) instead so the tail chain is
   short; odd heads shift to partitions 64..127 via SBUF->SBUF DMA.
 - partial = (O^T).T @ Wo rows in bf16, PSUM -> SBUF -> DRAM. The last
   chunk's Wo is split: m=0 half staged to SBUF during the final pair's
   attention, m=1 half + DVE add + DMA after the last norm.
"""

import os
import numpy as np

B, T, E, H = 2, 2048, 1024, 16
P = 128
NCORES = 8
G = 4            # head groups (tensor parallel)
HG = H // G      # heads per group = 4
D = E // H       # 64
CW = HG * D      # channels per group = 256
ET = E // P      # 8 e-tiles
AP2 = ET // 2    # 4 e-tile pairs
MT = CW // P     # 2 hd-tiles
TT = T // P      # 16 t-tiles
NQ = 512         # q-chunk width
QC = T // NQ     # 4 q-chunks
NEG = -1.0e9
SCALE = 1.0 / np.sqrt(E).astype(np.float32)  # 1/32

PH2 = True           # fp8-DR V (tiles>=4) + fp8-DR diag AV (qc>=1)

_CACHE: dict = {}


def _build_bass(debug_taps=False):
    import concourse.bass as bass
    import concourse.mybir as mybir
    import concourse.tile as tile
    from concourse import bacc

    f32 = mybir.dt.float32
    bf16 = mybir.dt.bfloat16
    f8 = mybir.dt.float8e4
    DR = mybir.MatmulPerfMode.DoubleRow
    Exp = mybir.ActivationFunctionType.Exp

    nc = bacc.Bacc("TRN2", target_bir_lowering=False, name="attn_tp")
    dbg = {}
    if debug_taps:
        dbg["qt00"] = nc.dram_tensor("dbg_qt00", [P, NQ], bf16, kind="ExternalOutput")
        dbg["kt00"] = nc.dram_tensor("dbg_kt00", [P, NQ], bf16, kind="ExternalOutput")
        dbg["vo0"] = nc.dram_tensor("dbg_vo0", [P, HG * (D + 1)], bf16, kind="ExternalOutput")
        dbg["u000"] = nc.dram_tensor("dbg_u000", [P, 2 * NQ], bf16, kind="ExternalOutput")
        dbg["ot00"] = nc.dram_tensor("dbg_ot00", [P, NQ], bf16, kind="ExternalOutput")
    XTBW = NQ if PH2 else T
    xt = nc.dram_tensor("xt", [P, ET, T], f8, kind="ExternalInput")
    xtb = nc.dram_tensor("xtb", [P, ET, XTBW], bf16, kind="ExternalInput")
    wq = nc.dram_tensor("wq", [P, ET, CW], f8, kind="ExternalInput")
    wk = nc.dram_tensor("wk", [P, ET, CW], f8, kind="ExternalInput")
    wv = nc.dram_tensor("wv", [P, ET, CW], bf16, kind="ExternalInput")
    wv8 = nc.dram_tensor("wv8", [P, ET, CW], f8, kind="ExternalInput")
    bq = nc.dram_tensor("bq", [P, MT], f32, kind="ExternalInput")
    wo = nc.dram_tensor("wo", [P, MT, E], bf16, kind="ExternalInput")
    tn = nc.dram_tensor("tn", [P, P], f32, kind="ExternalInput")
    out = nc.dram_tensor("out", [T, E], f32, kind="ExternalOutput")

    NVB = 4 if PH2 else TT   # number of bf16 V tiles

    with tile.TileContext(nc) as tc:
        with (
            tc.tile_pool(name="persist", bufs=1) as pers,
            tc.tile_pool(name="pp", bufs=2, space="PSUM") as pp,
            tc.tile_pool(name="ep", bufs=2, space="PSUM") as ep,
            tc.tile_pool(name="op", bufs=2, space="PSUM") as op,
            tc.tile_pool(name="up", bufs=6) as up,
            tc.tile_pool(name="ub", bufs=4) as ub,
            tc.tile_pool(name="sm", bufs=4) as sm,
            tc.tile_pool(name="ost", bufs=6) as ost,
        ):
            # ---- persistent SBUF tensors ----
            xt_sb = pers.tile([P, ET, T], f8, tag="xt_sb", name="xt_sb")
            xtb_sb = pers.tile([P, ET, XTBW], bf16, tag="xtb_sb", name="xtb_sb")
            wq_sb = pers.tile([P, ET, CW], f8, tag="wq_sb", name="wq_sb")
            wk_sb = pers.tile([P, ET, CW], f8, tag="wk_sb", name="wk_sb")
            wv_sb = pers.tile([P, ET, CW], bf16, tag="wv_sb", name="wv_sb")
            wv8_sb = pers.tile([P, ET, CW], f8, tag="wv8_sb", name="wv8_sb")
            wo_sb = pers.tile([P, MT, E], bf16, tag="wo_sb", name="wo_sb")
            bq_sb = pers.tile([P, MT], f32, tag="bq_sb", name="bq_sb")
            tn_sb = pers.tile([P, P], f32, tag="tn_sb", name="tn_sb")
            qt_t = [[pers.tile([P, NQ], bf16, tag=f"qt{m}_{n}", name=f"qt{m}_{n}")
                     for n in range(QC)] for m in range(MT)]
            kt_t = [[pers.tile([P, NQ], bf16, tag=f"kt{m}_{n}", name=f"kt{m}_{n}")
                     for n in range(QC)] for m in range(MT)]
            vo_bf = [pers.tile([P, HG, P], bf16, tag=f"vb{t}", name=f"vb{t}")
                     for t in range(NVB)]
            vop_t = [pers.tile([P, 2, HG, P], f8, tag=f"vo{tp}", name=f"vo{tp}")
                     for tp in range(TT // 2)]
            ot_t = [[pers.tile([P, NQ], bf16, tag=f"ot{m}_{n}", name=f"ot{m}_{n}")
                     for n in range(QC)] for m in range(MT)]
            wo_stage = [pers.tile([P, NQ], f32, tag=f"ws{g}", name=f"ws{g}")
                        for g in range(8)]

            # stationary V layout per (kt, head): parity-split so the AV
            # output lands where the ot tile wants it with no partition
            # shift: even heads [V 0:64 | ones@64 | zeros], odd heads
            # [ones@0 | zeros | V 64:128]. The ones row makes the DR matmul
            # emit the softmax denominator (even: PSUM row 64, odd: row 0).
            # Static ones/zero regions are initialized once on gpsimd.
            for tp in range(TT // 2):
                nc.gpsimd.memset(vop_t[tp][:, :, 0::2, D:D + 1], 1.0)
                nc.gpsimd.memset(vop_t[tp][:, :, 0::2, D + 1:], 0.0)
                nc.gpsimd.memset(vop_t[tp][:, :, 1::2, 0:1], 1.0)
                nc.gpsimd.memset(vop_t[tp][:, :, 1::2, 1:D], 0.0)
            for t in range(NVB):
                nc.gpsimd.memset(vo_bf[t][:, 0::2, D:D + 1], 1.0)
                nc.gpsimd.memset(vo_bf[t][:, 0::2, D + 1:], 0.0)
                nc.gpsimd.memset(vo_bf[t][:, 1::2, 0:1], 1.0)
                nc.gpsimd.memset(vo_bf[t][:, 1::2, 1:D], 0.0)

            # ---- input DMAs ----
            # weights first so the first projection starts early; x arrives
            # in column chunks so chunk-n projections do not wait on full x
            nc.scalar.dma_start(out=wq_sb, in_=wq[:, :, :])
            nc.scalar.dma_start(out=wk_sb, in_=wk[:, :, :])
            nc.scalar.dma_start(out=bq_sb, in_=bq[:, :])
            # chunk-0 columns split by e-tile pair so the first projection
            # matmul starts after the first quarter arrives
            for i in range(AP2):
                nc.sync.dma_start(out=xt_sb[:, 2 * i:2 * i + 2, 0:NQ],
                                  in_=xt[:, 2 * i:2 * i + 2, 0:NQ])
            nc.scalar.dma_start(out=wv_sb, in_=wv[:, :, :])
            nc.scalar.dma_start(out=wv8_sb, in_=wv8[:, :, :])
            nc.sync.dma_start(out=xtb_sb[:, :, 0:XTBW], in_=xtb[:, :, 0:XTBW])
            for n in range(1, QC):
                nc.sync.dma_start(out=xt_sb[:, :, n * NQ:(n + 1) * NQ],
                                  in_=xt[:, :, n * NQ:(n + 1) * NQ])
                if not PH2 and n < QC:
                    nc.scalar.dma_start(out=xtb_sb[:, :, n * NQ:(n + 1) * NQ],
                                        in_=xtb[:, :, n * NQ:(n + 1) * NQ])
            nc.scalar.dma_start(out=tn_sb, in_=tn[:, :])
            nc.scalar.dma_start(out=wo_sb, in_=wo[:, :, :])

            # ---- software-pipelined emission ----
            # PE engine queues are in-order, so attention batches (gated on
            # ACT exp) are interleaved with independent filler work: the next
            # chunk's projection groups and the previous chunk's Wo groups.

            def proj_closures(n):
                def qk_group(wsb, dst, m, biased):
                    def f():
                        ps = pp.tile([P, NQ], f32, tag="pp_t", name="psqk")
                        for i in range(AP2):
                            nc.tensor.matmul(
                                ps,
                                lhsT=wsb[:, 2 * i:2 * i + 2, m * P:(m + 1) * P],
                                rhs=xt_sb[:, 2 * i:2 * i + 2,
                                          n * NQ:(n + 1) * NQ],
                                start=(i == 0), stop=(i == AP2 - 1),
                                perf_mode=DR,
                            )
                        if biased:
                            nc.vector.tensor_scalar_add(
                                out=dst[m][n], in0=ps, scalar1=bq_sb[:, m:m + 1])
                        else:
                            nc.vector.tensor_copy(out=dst[m][n], in_=ps)
                    return f

                def v_group(t):
                    def f():
                        tp, j = t // 2, t % 2
                        psv = pp.tile([P, NQ], f32, tag="pp_t", name="psv")
                        if t < NVB:
                            for a in range(ET):
                                nc.tensor.matmul(
                                    psv[:, :CW],
                                    lhsT=xtb_sb[:, a, t * P:(t + 1) * P],
                                    rhs=wv_sb[:, a, :],
                                    start=(a == 0), stop=(a == ET - 1),
                                )
                            pv = psv[:, :CW].rearrange("p (h d) -> p h d", h=HG)
                            nc.vector.tensor_copy(
                                out=vo_bf[t][:, 0::2, 0:D], in_=pv[:, 0::2, :])
                            nc.vector.tensor_copy(
                                out=vo_bf[t][:, 1::2, D:P], in_=pv[:, 1::2, :])
                            # gpsimd cannot read PSUM: cast the fp8 copy from
                            # the bf16 SBUF tile instead
                            nc.gpsimd.tensor_copy(
                                out=vop_t[tp][:, j, :, :],
                                in_=vo_bf[t][:, :, :])
                        else:
                            for i in range(AP2):
                                nc.tensor.matmul(
                                    psv[:, :CW],
                                    lhsT=xt_sb[:, 2 * i:2 * i + 2, t * P:(t + 1) * P],
                                    rhs=wv8_sb[:, 2 * i:2 * i + 2, :],
                                    start=(i == 0), stop=(i == AP2 - 1),
                                    perf_mode=DR,
                                )
                            pv = psv[:, :CW].rearrange("p (h d) -> p h d", h=HG)
                            nc.vector.tensor_copy(
                                out=vop_t[tp][:, j, 0::2, 0:D], in_=pv[:, 0::2, :])
                            nc.vector.tensor_copy(
                                out=vop_t[tp][:, j, 1::2, D:P], in_=pv[:, 1::2, :])
                    return f

                fs = []
                for m in range(MT):
                    fs.append(qk_group(wq_sb, qt_t, m, True))
                    fs.append(qk_group(wk_sb, kt_t, m, False))
                for t in range(4 * n, 4 * n + 4):
                    fs.append(v_group(t))
                return fs

            def wo_closures(qc):
                def wo_group(ti, ec):
                    def f():
                        wp = pp.tile([P, NQ], f32, tag="pp_t", name="wp")
                        for m in range(MT):
                            nc.tensor.matmul(
                                wp,
                                lhsT=ot_t[m][qc][:, (ti % 4) * P:(ti % 4 + 1) * P],
                                rhs=wo_sb[:, m, ec * NQ:(ec + 1) * NQ],
                                start=(m == 0), stop=(m == MT - 1),
                            )
                        so = ost.tile([P, NQ], f32, tag="ost", name="so")
                        if ec % 2 == 0:
                            nc.scalar.copy(out=so, in_=wp)
                        else:
                            nc.vector.tensor_copy(out=so, in_=wp)
                        nc.sync.dma_start(
                            out=out[ti * P:(ti + 1) * P, ec * NQ:(ec + 1) * NQ], in_=so)
                    return f
                return [wo_group(ti, ec)
                        for ti in range(4 * qc, 4 * qc + 4) for ec in range(E // NQ)]

            def wo_m0_closures(qc):
                # first-half contraction (heads 0/1), staged to SBUF f32;
                # runs hidden inside the final pair's attention
                def g(ti, ec):
                    def f():
                        wp = pp.tile([P, NQ], f32, tag="pp_t", name="wp0")
                        nc.tensor.matmul(
                            wp,
                            lhsT=ot_t[0][qc][:, (ti % 4) * P:(ti % 4 + 1) * P],
                            rhs=wo_sb[:, 0, ec * NQ:(ec + 1) * NQ],
                            start=True, stop=True)
                        s = wo_stage[(ti % 4) * 2 + ec]
                        if ec % 2 == 0:
                            nc.scalar.copy(out=s, in_=wp)
                        else:
                            nc.vector.tensor_copy(out=s, in_=wp)
                    return f
                return [g(ti, ec)
                        for ti in range(4 * qc, 4 * qc + 4) for ec in range(E // NQ)]

            def wo_m1_closures(qc):
                def g(ti, ec):
                    def f():
                        wp = pp.tile([P, NQ], f32, tag="pp_t", name="wp1")
                        nc.tensor.matmul(
                            wp,
                            lhsT=ot_t[1][qc][:, (ti % 4) * P:(ti % 4 + 1) * P],
                            rhs=wo_sb[:, 1, ec * NQ:(ec + 1) * NQ],
                            start=True, stop=True)
                        so = ost.tile([P, NQ], f32, tag="ost", name="so")
                        nc.vector.tensor_add(
                            so, wp, wo_stage[(ti % 4) * 2 + ec])
                        nc.sync.dma_start(
                            out=out[ti * P:(ti + 1) * P, ec * NQ:(ec + 1) * NQ], in_=so)
                    return f
                return [g(ti, ec)
                        for ti in range(4 * qc, 4 * qc + 4) for ec in range(E // NQ)]

            def pair_stream(qc, pair):
                nkt = 4 * qc + 4
                dr_all = PH2 and qc >= 1
                o_ps = {}

                def alloc():
                    for h in pair:
                        o_ps[h] = op.tile([P, NQ], f32, tag="o_ps", name=f"o_ps{h}")

                def ebatch(ktb, u_ts):
                    kts = (ktb, ktb + 1)
                    offs = [max(0, (kt - 4 * qc) * P) for kt in kts]
                    off0, off1 = offs
                    diag = ktb >= 4 * qc
                    e_ts = {}
                    for h in pair:
                        e_ts[h] = ep.tile([P, 2 * NQ], f32, tag="e_ps",
                                          name=f"e_ps{h}")
                    # alternate heads so adjacent matmuls use disjoint PE
                    # row groups (base partitions 0/64): the 64x128 row tiles
                    # run the two heads' streams concurrently
                    for j, kt in enumerate(kts):
                        eoff = offs[j]
                        for h in pair:
                            m, r0 = h // 2, 64 * (h % 2)
                            nc.tensor.matmul(
                                e_ts[h][:, j * NQ + eoff:(j + 1) * NQ],
                                lhsT=kt_t[m][kt // 4][r0:r0 + D,
                                                      (kt % 4) * P:(kt % 4 + 1) * P],
                                rhs=qt_t[m][qc][r0:r0 + D, eoff:NQ],
                                start=True, stop=True,
                            )
                    for h in pair:
                        if diag:
                            # additive causal mask on PSUM: j=0 diag block,
                            # then j=1 diag block
                            nc.vector.tensor_add(
                                e_ts[h][:, off0:off0 + P],
                                e_ts[h][:, off0:off0 + P], tn_sb)
                            nc.vector.tensor_add(
                                e_ts[h][:, NQ + off1:NQ + off1 + P],
                                e_ts[h][:, NQ + off1:NQ + off1 + P], tn_sb)
                            if dr_all:
                                # fp8 U consumed by a DR matmul: one exp over
                                # the whole span (the dead strip exps psum
                                # garbage), then zero the strip so the DR
                                # stream multiplies zeros there
                                ut = up.tile([P, 2 * NQ], f8, tag="u", name=f"u{h}")
                                u_ts[h] = ut
                                nc.scalar.activation(
                                    ut[:, off0:], e_ts[h][:, off0:],
                                    Exp, scale=float(SCALE))
                                nc.gpsimd.memset(ut[:, NQ + off0:NQ + off1], 0.0)
                                continue
                            ut = ub.tile([P, 2 * NQ], bf16, tag="ub", name=f"ub{h}")
                        else:
                            ut = up.tile([P, 2 * NQ], f8, tag="u", name=f"u{h}")
                        u_ts[h] = ut
                        nc.scalar.activation(
                            ut[:, off0:], e_ts[h][:, off0:],
                            Exp, scale=float(SCALE))
                    if debug_taps and qc == 0 and pair == (0, 1) and ktb == 0:
                        nc.sync.dma_start(out=dbg["u000"][:, :], in_=u_ts[0])

                def avbatch(ktb, u_ts):
                    offs = [max(0, (kt - 4 * qc) * P) for kt in (ktb, ktb + 1)]
                    off0 = offs[0]
                    diag = ktb >= 4 * qc
                    tp = ktb // 2
                    for h in pair:
                        if dr_all or not diag:
                            uv = u_ts[h].rearrange("p (j q) -> p j q", j=2)
                            stop_kt = (4 * qc - 2) if not dr_all else (nkt - 2)
                            nc.tensor.matmul(
                                o_ps[h][:, off0:NQ],
                                lhsT=vop_t[tp][:, :, h, :],
                                rhs=uv[:, :, off0:NQ],
                                start=(ktb == 0), stop=(ktb == stop_kt),
                                perf_mode=DR,
                                skip_group_check=True,
                            )
                        else:
                            for j, kt in enumerate((ktb, ktb + 1)):
                                off = offs[j]
                                nc.tensor.matmul(
                                    o_ps[h][:, off:NQ],
                                    lhsT=vo_bf[kt][:, h, :],
                                    rhs=u_ts[h][:, j * NQ + off:(j + 1) * NQ],
                                    start=(kt == 0),
                                    stop=(kt == nkt - 1),
                                    skip_group_check=True,
                                )

                def norm(h):
                    # hw partition_broadcast reads partition 0 regardless of
                    # the input AP offset (and the custom reciprocal is only
                    # reliable at partition 0). Even heads: V sums at PSUM
                    # rows 0:64, denominator at row 64, DMA-shifted to row 0.
                    # Odd heads: denominator already at row 0, V at 64:128,
                    # no shifts, and the mul writes ot rows 64:128 in-lane.
                    m = h // 2
                    # snapshot the accumulator to SBUF first: a [P,NQ] DVE
                    # copy costs the same as a single-row copy (per-lane
                    # throughput) and releases the PSUM buffer for the next
                    # pair immediately instead of after the whole
                    # recip/broadcast chain. The custom reciprocal (and the
                    # gpsimd broadcast source) must sit at partition 0.
                    og = sm.tile([P, NQ], f32, tag="og", name="og")
                    nc.vector.tensor_copy(out=og, in_=o_ps[h])
                    dn = sm.tile([P, NQ], f32, tag="dn", name="dn")
                    rc = sm.tile([P, NQ], f32, tag="rc", name="rc")
                    bc = sm.tile([P, NQ], f32, tag="bc", name="bc")
                    if h % 2 == 0:
                        nc.sync.dma_start(out=dn[0:1, :], in_=og[D:D + 1, :])
                        nc.vector.reciprocal_approx_fast(out=rc[0:1, :], in_=dn[0:1, :])
                        nc.gpsimd.partition_broadcast(bc[0:D, :], rc[0:1, :], channels=D)
                        nc.vector.tensor_mul(
                            ot_t[m][qc][0:D, :], og[0:D, :], bc[0:D, :])
                    else:
                        nc.vector.reciprocal_approx_fast(out=rc[0:1, :], in_=og[0:1, :])
                        # broadcast ignores the output partition offset, so
                        # fill all 128 partitions and use the top half
                        nc.gpsimd.partition_broadcast(bc[:, :], rc[0:1, :], channels=P)
                        nc.vector.tensor_mul(
                            ot_t[m][qc][D:P, :], og[D:P, :], bc[D:P, :])

                alloc()
                for ktb in range(0, nkt, 2):
                    u_ts = {}
                    yield (lambda ktb=ktb, u_ts=u_ts: ebatch(ktb, u_ts))
                    yield (lambda ktb=ktb, u_ts=u_ts: avbatch(ktb, u_ts))
                for h in pair:
                    yield (lambda h=h: norm(h))

            def run_slots(slots, fillers):
                # distribute fillers across the attention slots only (the
                # last two slots are the norms): every filler's engine ops
                # must precede the norm chain, because the norm muls wait on
                # the slow gpsimd broadcast and any PE-feeding DVE op behind
                # them (the next chunk's qt/kt evac) would head-of-line
                # block on the in-order DVE queue
                nf, ns, fi = len(fillers), max(len(slots) - 2, 1), 0
                for i, sf in enumerate(slots):
                    if i >= ns:
                        while fi < nf:
                            fillers[fi]()
                            fi += 1
                    sf()
                    want = min((i + 1) * nf // ns, nf)
                    while fi < want:
                        fillers[fi]()
                        fi += 1

            # emit only what the first attention pair needs up front (heads
            # 0/1 projections + the first two V tiles); the rest interleaves
            # into the first pair's exp-gated gaps
            pc0 = proj_closures(0)
            for f in (pc0[0], pc0[1], pc0[4], pc0[5]):
                f()
            pre = [pc0[6], pc0[7], pc0[2], pc0[3]]
            for qc in range(QC):
                fillers = []
                if qc == 0:
                    fillers += pre
                if qc + 1 < QC:
                    fillers += proj_closures(qc + 1)
                if qc >= 1:
                    fillers += wo_closures(qc - 1)
                s1 = list(pair_stream(qc, (0, 1)))
                s2 = list(pair_stream(qc, (2, 3)))
                half = len(fillers) // 2
                f1, f2 = fillers[:half], fillers[half:]
                if qc == QC - 1:
                    # heads 0/1 are normalized by the end of s1, so the m=0
                    # Wo half hides inside the final pair's attention
                    f2 = f2 + wo_m0_closures(qc)
                run_slots(s1, f1)
                run_slots(s2, f2)
            for f in wo_m1_closures(QC - 1):
                f()
            if debug_taps:
                nc.sync.dma_start(out=dbg["qt00"][:, :], in_=qt_t[0][0])
                nc.sync.dma_start(out=dbg["kt00"][:, :], in_=kt_t[0][0])
                nc.sync.dma_start(out=dbg["vo0"][:, :],
                                  in_=vo_bf[0].rearrange("p h d -> p (h d)"))
                nc.sync.dma_start(out=dbg["ot00"][:, :], in_=ot_t[0][0])
    nc.compile()
    return nc


def _prepare_in_maps(x, Wq, bq, Wk, Wv, Wo):
    import ml_dtypes
    bfd = ml_dtypes.bfloat16
    f8d = ml_dtypes.float8_e4m3fn
    tn = np.tril(np.full((P, P), NEG, np.float32), -1)
    xl = [np.ascontiguousarray(
        x[b].T.reshape(ET, P, T).transpose(1, 0, 2)) for b in range(B)]
    xtb8 = [a.astype(f8d) for a in xl]
    if PH2:
        xtbb = [np.ascontiguousarray(a[:, :, :NQ]).astype(bfd) for a in xl]
    else:
        xtbb = [a.astype(bfd) for a in xl]
    in_maps = []
    for c in range(NCORES):
        b, g = c // G, c % G
        cs = slice(g * CW, (g + 1) * CW)
        bq_g = np.ascontiguousarray(bq[cs].reshape(MT, P).T)
        def wlay(w, dt):  # [E, CW] -> [P, ET, CW] with e = a*P + p
            return np.ascontiguousarray(
                w.reshape(ET, P, CW).transpose(1, 0, 2)).astype(dt)
        wo_l = np.ascontiguousarray(
            Wo[cs, :].reshape(MT, P, E).transpose(1, 0, 2)).astype(bfd)
        in_maps.append({
            "xt": xtb8[b],
            "xtb": xtbb[b],
            "wq": wlay(Wq[:, cs], f8d),
            "wk": wlay(Wk[:, cs], f8d),
            "wv": wlay(Wv[:, cs], bfd),
            "wv8": wlay(Wv[:, cs], f8d),
            "bq": bq_g,
            "wo": wo_l,
            "tn": tn,
        })
    return in_maps


def _run(inputs, trace=False, trace_kwargs=None, debug_taps=False):
    from concourse.bass_utils import run_bass_kernel_spmd

    key = ("nc", debug_taps)
    if key not in _CACHE:
        _CACHE[key] = _build_bass(debug_taps=debug_taps)
    nc = _CACHE[key]

    x = np.asarray(inputs["x"], np.float32)
    Wq = np.asarray(inputs["Wq"], np.float32)
    Wk = np.asarray(inputs["Wk"], np.float32)
    Wv = np.asarray(inputs["Wv"], np.float32)
    Wo = np.asarray(inputs["Wo"], np.float32)
    bq = np.asarray(inputs["bq"], np.float32)
    bv = np.asarray(inputs["bv"], np.float32)
    bo = np.asarray(inputs["bo"], np.float32)

    in_maps = _prepare_in_maps(x, Wq, bq, Wk, Wv, Wo)
    res = run_bass_kernel_spmd(
        nc, in_maps, core_ids=list(range(NCORES)),
        trace=trace, **(trace_kwargs or {}))

    bias_row = (bv @ Wo + bo).astype(np.float32)
    y = np.empty((B, T, E), np.float32)
    for b in range(B):
        acc = res.results[G * b]["out"].astype(np.float32).copy()
        for g in range(1, G):
            acc += res.results[G * b + g]["out"]
        y[b] = acc + bias_row
    return y, res


def kernel(**inputs) -> np.ndarray:
    return _run(inputs, trace=False)[0]


# revision 27
# speedup vs baseline: 1.1911x; 1.0329x over previous
"""Causal multi-head attention layer for Trainium2, sharded over 8 NeuronCores.

Problem: B=2, T=2048, E=1024, H=16 heads (D=64), fp32.
  out = softmax(mask(QK^T)/sqrt(E)) V Wo + bo   with Q=xWq+bq etc.

Sharding: data-parallel over batch (2) x tensor-parallel over head groups (4):
core c -> batch b=c//4, head group g=c%4 (4 heads, 256 channels).
Each core computes partial = attn_heads(x_b) @ Wo[rows of g]; host sums the 4
partials per batch and adds the bias row.

Math folds used (all exact):
 - bk drops out of softmax (additive shift along the softmax axis).
 - attn rows sum to 1  =>  attn @ (V + 1 bv^T) = attn@V + 1 bv^T, so bv enters
   the output as the constant row bv @ Wo, added on the host with bo.
 - bq is added to Q^T on-chip (per-partition bias).

Measured PE law (trace): effective clock duty-cycles ~1.2-2.4GHz (power
throttle), bf16/fp8 stream 1 col/cycle, fp8e4 DoubleRow streams 2 phys cols
per cycle for a 256-deep contraction. So DR fp8 ~2x bf16 throughput; plain
fp8 = bf16 speed.

Precision: fp8 error concentrates in rows whose softmax averages few keys
(early rows). Chunk 0 (q<512) keeps a bf16 V + bf16 diag path; chunks 1-3
run the causal-diagonal kt-pairs in fp8 DR like the off-diagonal (those rows
attend >=512 keys). V is projected in fp8 DR for key tiles 4..15 and bf16 for
tiles 0..3.

Device layout per core (matmuls: out = lhsT.T @ rhs, contraction on
partitions):
 - Q^T/K^T = W^T x^T in fp8 DoubleRow over e-tile pairs -> bf16 [ch, t].
 - V tiles 0..3: bf16 x W -> vo_bf [k, h, D+1] (ones col for the denominator)
   + fp8 cast into vop; tiles 4..15: fp8 DR -> vop [k, ktpair, h, 128-pad]
   directly (ones row at 64, zero pad above).
 - energy^T = K Q^T per head, bf16, d=64 contraction at PE row base 0/64
   (64x128 row tiling runs the two heads of a pair concurrently); causal
   mask added (-1e9) on PSUM before exp.
 - exp on ACT: chunk 0 diag pairs -> bf16 U; everything else -> fp8 U (diag
   pairs split j0/j1 + gpsimd memset of the masked gap so DR sees zeros).
 - O^T accumulates per head in PSUM: one fp8 DR matmul per kt-pair (V rows
   0..63, ones denominator row 64, zero pad above); chunk 0 diag in bf16
   M=65 matmuls.
 - normalize with DVE mult by gpsimd-broadcast reciprocal; the final pair
   uses a PE outer-product broadcast (fp32r) instead so the tail chain is
   short; odd heads shift to partitions 64..127 via SBUF->SBUF DMA.
 - partial = (O^T).T @ Wo rows in bf16, PSUM -> SBUF -> DRAM. The last
   chunk's Wo is split: m=0 half staged to SBUF during the final pair's
   attention, m=1 half + DVE add + DMA after the last norm.
"""

import os
import numpy as np

B, T, E, H = 2, 2048, 1024, 16
P = 128
NCORES = 8
G = 4            # head groups (tensor parallel)
HG = H // G      # heads per group = 4
D = E // H       # 64
CW = HG * D      # channels per group = 256
ET = E // P      # 8 e-tiles
AP2 = ET // 2    # 4 e-tile pairs
MT = CW // P     # 2 hd-tiles
TT = T // P      # 16 t-tiles
NQ = 512         # q-chunk width
QC = T // NQ     # 4 q-chunks
NEG = -1.0e9
SCALE = 1.0 / np.sqrt(E).astype(np.float32)  # 1/32

PH2 = True           # fp8-DR V (tiles>=4) + fp8-DR diag AV (qc>=1)

_CACHE: dict = {}


def _build_bass(debug_taps=False):
    import concourse.bass as bass
    import concourse.mybir as mybir
    import concourse.tile as tile
    from concourse import bacc

    f32 = mybir.dt.float32
    bf16 = mybir.dt.bfloat16
    f8 = mybir.dt.float8e4
    DR = mybir.MatmulPerfMode.DoubleRow
    Exp = mybir.ActivationFunctionType.Exp

    nc = bacc.Bacc("TRN2", target_bir_lowering=False, name="attn_tp")
    dbg = {}
    if debug_taps:
        dbg["qt00"] = nc.dram_tensor("dbg_qt00", [P, NQ], bf16, kind="ExternalOutput")
        dbg["kt00"] = nc.dram_tensor("dbg_kt00", [P, NQ], bf16, kind="ExternalOutput")
        dbg["vo0"] = nc.dram_tensor("dbg_vo0", [P, HG * (D + 1)], bf16, kind="ExternalOutput")
        dbg["u000"] = nc.dram_tensor("dbg_u000", [P, 2 * NQ], bf16, kind="ExternalOutput")
        dbg["ot00"] = nc.dram_tensor("dbg_ot00", [P, NQ], bf16, kind="ExternalOutput")
    XTBW = NQ if PH2 else T
    xt = nc.dram_tensor("xt", [P, ET, T], f8, kind="ExternalInput")
    xtb = nc.dram_tensor("xtb", [P, ET, XTBW], bf16, kind="ExternalInput")
    wq = nc.dram_tensor("wq", [P, ET, CW], f8, kind="ExternalInput")
    wk = nc.dram_tensor("wk", [P, ET, CW], f8, kind="ExternalInput")
    wv = nc.dram_tensor("wv", [P, ET, CW], bf16, kind="ExternalInput")
    wv8 = nc.dram_tensor("wv8", [P, ET, CW], f8, kind="ExternalInput")
    bq = nc.dram_tensor("bq", [P, MT], f32, kind="ExternalInput")
    wo = nc.dram_tensor("wo", [P, MT, E], bf16, kind="ExternalInput")
    tn = nc.dram_tensor("tn", [P, P], f32, kind="ExternalInput")
    out = nc.dram_tensor("out", [T, E], f32, kind="ExternalOutput")

    NVB = 4 if PH2 else TT   # number of bf16 V tiles

    with tile.TileContext(nc) as tc:
        with (
            tc.tile_pool(name="persist", bufs=1) as pers,
            tc.tile_pool(name="pp", bufs=2, space="PSUM") as pp,
            tc.tile_pool(name="ep", bufs=2, space="PSUM") as ep,
            tc.tile_pool(name="op", bufs=2, space="PSUM") as op,
            tc.tile_pool(name="up", bufs=6) as up,
            tc.tile_pool(name="ub", bufs=4) as ub,
            tc.tile_pool(name="sm", bufs=4) as sm,
            tc.tile_pool(name="ost", bufs=6) as ost,
        ):
            # ---- persistent SBUF tensors ----
            xt_sb = pers.tile([P, ET, T], f8, tag="xt_sb", name="xt_sb")
            xtb_sb = pers.tile([P, ET, XTBW], bf16, tag="xtb_sb", name="xtb_sb")
            wq_sb = pers.tile([P, ET, CW], f8, tag="wq_sb", name="wq_sb")
            wk_sb = pers.tile([P, ET, CW], f8, tag="wk_sb", name="wk_sb")
            wv_sb = pers.tile([P, ET, CW], bf16, tag="wv_sb", name="wv_sb")
            wv8_sb = pers.tile([P, ET, CW], f8, tag="wv8_sb", name="wv8_sb")
            wo_sb = pers.tile([P, MT, E], bf16, tag="wo_sb", name="wo_sb")
            bq_sb = pers.tile([P, MT], f32, tag="bq_sb", name="bq_sb")
            tn_sb = pers.tile([P, P], f32, tag="tn_sb", name="tn_sb")
            qt_t = [[pers.tile([P, NQ], bf16, tag=f"qt{m}_{n}", name=f"qt{m}_{n}")
                     for n in range(QC)] for m in range(MT)]
            kt_t = [[pers.tile([P, NQ], bf16, tag=f"kt{m}_{n}", name=f"kt{m}_{n}")
                     for n in range(QC)] for m in range(MT)]
            vo_bf = [pers.tile([P, HG, P], bf16, tag=f"vb{t}", name=f"vb{t}")
                     for t in range(NVB)]
            vop_t = [pers.tile([P, 2, HG, P], f8, tag=f"vo{tp}", name=f"vo{tp}")
                     for tp in range(TT // 2)]
            ot_t = [[pers.tile([P, NQ], bf16, tag=f"ot{m}_{n}", name=f"ot{m}_{n}")
                     for n in range(QC)] for m in range(MT)]
            wo_stage = [pers.tile([P, NQ], f32, tag=f"ws{g}", name=f"ws{g}")
                        for g in range(8)]

            # stationary V layout per (kt, head): parity-split so the AV
            # output lands where the ot tile wants it with no partition
            # shift: even heads [V 0:64 | ones@64 | zeros], odd heads
            # [ones@0 | zeros | V 64:128]. The ones row makes the DR matmul
            # emit the softmax denominator (even: PSUM row 64, odd: row 0).
            # Static ones/zero regions are initialized once on gpsimd.
            for tp in range(TT // 2):
                nc.gpsimd.memset(vop_t[tp][:, :, 0::2, D:D + 1], 1.0)
                nc.gpsimd.memset(vop_t[tp][:, :, 0::2, D + 1:], 0.0)
                nc.gpsimd.memset(vop_t[tp][:, :, 1::2, 0:1], 1.0)
                nc.gpsimd.memset(vop_t[tp][:, :, 1::2, 1:D], 0.0)
            for t in range(NVB):
                nc.gpsimd.memset(vo_bf[t][:, 0::2, D:D + 1], 1.0)
                nc.gpsimd.memset(vo_bf[t][:, 0::2, D + 1:], 0.0)
                nc.gpsimd.memset(vo_bf[t][:, 1::2, 0:1], 1.0)
                nc.gpsimd.memset(vo_bf[t][:, 1::2, 1:D], 0.0)

            # ---- input DMAs ----
            # weights first so the first projection starts early; x arrives
            # in column chunks so chunk-n projections do not wait on full x
            nc.scalar.dma_start(out=wq_sb, in_=wq[:, :, :])
            nc.scalar.dma_start(out=wk_sb, in_=wk[:, :, :])
            nc.scalar.dma_start(out=bq_sb, in_=bq[:, :])
            # chunk-0 columns split by e-tile pair so the first projection
            # matmul starts after the first quarter arrives
            for i in range(AP2):
                nc.sync.dma_start(out=xt_sb[:, 2 * i:2 * i + 2, 0:NQ],
                                  in_=xt[:, 2 * i:2 * i + 2, 0:NQ])
            nc.scalar.dma_start(out=wv_sb, in_=wv[:, :, :])
            nc.scalar.dma_start(out=wv8_sb, in_=wv8[:, :, :])
            nc.sync.dma_start(out=xtb_sb[:, :, 0:XTBW], in_=xtb[:, :, 0:XTBW])
            for n in range(1, QC):
                nc.sync.dma_start(out=xt_sb[:, :, n * NQ:(n + 1) * NQ],
                                  in_=xt[:, :, n * NQ:(n + 1) * NQ])
                if not PH2 and n < QC:
                    nc.scalar.dma_start(out=xtb_sb[:, :, n * NQ:(n + 1) * NQ],
                                        in_=xtb[:, :, n * NQ:(n + 1) * NQ])
            nc.scalar.dma_start(out=tn_sb, in_=tn[:, :])
            nc.scalar.dma_start(out=wo_sb, in_=wo[:, :, :])

            # ---- software-pipelined emission ----
            # PE engine queues are in-order, so attention batches (gated on
            # ACT exp) are interleaved with independent filler work: the next
            # chunk's projection groups and the previous chunk's Wo groups.

            def proj_closures(n):
                def qk_group(wsb, dst, m, biased):
                    def f():
                        ps = pp.tile([P, NQ], f32, tag="pp_t", name="psqk")
                        for i in range(AP2):
                            nc.tensor.matmul(
                                ps,
                                lhsT=wsb[:, 2 * i:2 * i + 2, m * P:(m + 1) * P],
                                rhs=xt_sb[:, 2 * i:2 * i + 2,
                                          n * NQ:(n + 1) * NQ],
                                start=(i == 0), stop=(i == AP2 - 1),
                                perf_mode=DR,
                            )
                        if biased:
                            nc.vector.tensor_scalar_add(
                                out=dst[m][n], in0=ps, scalar1=bq_sb[:, m:m + 1])
                        else:
                            nc.vector.tensor_copy(out=dst[m][n], in_=ps)
                    return f

                def v_group(t):
                    def f():
                        tp, j = t // 2, t % 2
                        psv = pp.tile([P, NQ], f32, tag="pp_t", name="psv")
                        if t < NVB:
                            for a in range(ET):
                                nc.tensor.matmul(
                                    psv[:, :CW],
                                    lhsT=xtb_sb[:, a, t * P:(t + 1) * P],
                                    rhs=wv_sb[:, a, :],
                                    start=(a == 0), stop=(a == ET - 1),
                                )
                            pv = psv[:, :CW].rearrange("p (h d) -> p h d", h=HG)
                            nc.vector.tensor_copy(
                                out=vo_bf[t][:, 0::2, 0:D], in_=pv[:, 0::2, :])
                            nc.vector.tensor_copy(
                                out=vo_bf[t][:, 1::2, D:P], in_=pv[:, 1::2, :])
                            # gpsimd cannot read PSUM: cast the fp8 copy from
                            # the bf16 SBUF tile instead
                            nc.gpsimd.tensor_copy(
                                out=vop_t[tp][:, j, :, :],
                                in_=vo_bf[t][:, :, :])
                        else:
                            for i in range(AP2):
                                nc.tensor.matmul(
                                    psv[:, :CW],
                                    lhsT=xt_sb[:, 2 * i:2 * i + 2, t * P:(t + 1) * P],
                                    rhs=wv8_sb[:, 2 * i:2 * i + 2, :],
                                    start=(i == 0), stop=(i == AP2 - 1),
                                    perf_mode=DR,
                                )
                            pv = psv[:, :CW].rearrange("p (h d) -> p h d", h=HG)
                            nc.vector.tensor_copy(
                                out=vop_t[tp][:, j, 0::2, 0:D], in_=pv[:, 0::2, :])
                            nc.vector.tensor_copy(
                                out=vop_t[tp][:, j, 1::2, D:P], in_=pv[:, 1::2, :])
                    return f

                fs = []
                for m in range(MT):
                    fs.append(qk_group(wq_sb, qt_t, m, True))
                    fs.append(qk_group(wk_sb, kt_t, m, False))
                for t in range(4 * n, 4 * n + 4):
                    fs.append(v_group(t))
                return fs

            def wo_closures(qc):
                def wo_group(ti, ec):
                    def f():
                        wp = pp.tile([P, NQ], f32, tag="pp_t", name="wp")
                        for m in range(MT):
                            nc.tensor.matmul(
                                wp,
                                lhsT=ot_t[m][qc][:, (ti % 4) * P:(ti % 4 + 1) * P],
                                rhs=wo_sb[:, m, ec * NQ:(ec + 1) * NQ],
                                start=(m == 0), stop=(m == MT - 1),
                            )
                        so = ost.tile([P, NQ], f32, tag="ost", name="so")
                        if ec % 2 == 0:
                            nc.scalar.copy(out=so, in_=wp)
                        else:
                            nc.vector.tensor_copy(out=so, in_=wp)
                        nc.sync.dma_start(
                            out=out[ti * P:(ti + 1) * P, ec * NQ:(ec + 1) * NQ], in_=so)
                    return f
                return [wo_group(ti, ec)
                        for ti in range(4 * qc, 4 * qc + 4) for ec in range(E // NQ)]

            def wo_m0_closures(qc):
                # first-half contraction (heads 0/1), staged to SBUF f32;
                # runs hidden inside the final pair's attention
                def g(ti, ec):
                    def f():
                        wp = pp.tile([P, NQ], f32, tag="pp_t", name="wp0")
                        nc.tensor.matmul(
                            wp,
                            lhsT=ot_t[0][qc][:, (ti % 4) * P:(ti % 4 + 1) * P],
                            rhs=wo_sb[:, 0, ec * NQ:(ec + 1) * NQ],
                            start=True, stop=True)
                        s = wo_stage[(ti % 4) * 2 + ec]
                        if ec % 2 == 0:
                            nc.scalar.copy(out=s, in_=wp)
                        else:
                            nc.vector.tensor_copy(out=s, in_=wp)
                    return f
                return [g(ti, ec)
                        for ti in range(4 * qc, 4 * qc + 4) for ec in range(E // NQ)]

            def wo_m1_closures(qc):
                def g(ti, ec):
                    def f():
                        wp = pp.tile([P, NQ], f32, tag="pp_t", name="wp1")
                        nc.tensor.matmul(
                            wp,
                            lhsT=ot_t[1][qc][:, (ti % 4) * P:(ti % 4 + 1) * P],
                            rhs=wo_sb[:, 1, ec * NQ:(ec + 1) * NQ],
                            start=True, stop=True)
                        so = ost.tile([P, NQ], f32, tag="ost", name="so")
                        nc.vector.tensor_add(
                            so, wp, wo_stage[(ti % 4) * 2 + ec])
                        nc.sync.dma_start(
                            out=out[ti * P:(ti + 1) * P, ec * NQ:(ec + 1) * NQ], in_=so)
                    return f
                return [g(ti, ec)
                        for ti in range(4 * qc, 4 * qc + 4) for ec in range(E // NQ)]

            def pair_stream(qc, pair):
                nkt = 4 * qc + 4
                dr_all = PH2 and qc >= 1
                o_ps = {}

                def alloc():
                    for h in pair:
                        o_ps[h] = op.tile([P, NQ], f32, tag="o_ps", name=f"o_ps{h}")

                def ebatch(ktb, u_ts):
                    kts = (ktb, ktb + 1)
                    offs = [max(0, (kt - 4 * qc) * P) for kt in kts]
                    off0, off1 = offs
                    diag = ktb >= 4 * qc
                    e_ts = {}
                    for h in pair:
                        e_ts[h] = ep.tile([P, 2 * NQ], f32, tag="e_ps",
                                          name=f"e_ps{h}")
                    # alternate heads so adjacent matmuls use disjoint PE
                    # row groups (base partitions 0/64): the 64x128 row tiles
                    # run the two heads' streams concurrently
                    for j, kt in enumerate(kts):
                        eoff = offs[j]
                        for h in pair:
                            m, r0 = h // 2, 64 * (h % 2)
                            nc.tensor.matmul(
                                e_ts[h][:, j * NQ + eoff:(j + 1) * NQ],
                                lhsT=kt_t[m][kt // 4][r0:r0 + D,
                                                      (kt % 4) * P:(kt % 4 + 1) * P],
                                rhs=qt_t[m][qc][r0:r0 + D, eoff:NQ],
                                start=True, stop=True,
                            )
                    for h in pair:
                        if diag:
                            # additive causal mask on PSUM: j=0 diag block,
                            # then j=1 diag block
                            base = e_ts[h][:, off0:off0 + P]
                            blk = bass.AP(
                                tensor=base.tensor,
                                offset=base.offset,
                                ap=[list(base.ap[0]), [NQ + P, 2],
                                    list(base.ap[1])],
                            )
                            nc.vector.tensor_add(
                                blk, blk,
                                tn_sb.unsqueeze(1).broadcast_to([P, 2, P]))
                            if dr_all:
                                # fp8 U consumed by a DR matmul: one exp over
                                # the whole span (the dead strip exps psum
                                # garbage), then zero the strip so the DR
                                # stream multiplies zeros there
                                ut = up.tile([P, 2 * NQ], f8, tag="u", name=f"u{h}")
                                u_ts[h] = ut
                                nc.scalar.activation(
                                    ut[:, off0:], e_ts[h][:, off0:],
                                    Exp, scale=float(SCALE))
                                nc.gpsimd.memset(ut[:, NQ + off0:NQ + off1], 0.0)
                                continue
                            ut = ub.tile([P, 2 * NQ], bf16, tag="ub", name=f"ub{h}")
                        else:
                            ut = up.tile([P, 2 * NQ], f8, tag="u", name=f"u{h}")
                        u_ts[h] = ut
                        nc.scalar.activation(
                            ut[:, off0:], e_ts[h][:, off0:],
                            Exp, scale=float(SCALE))
                    if debug_taps and qc == 0 and pair == (0, 1) and ktb == 0:
                        nc.sync.dma_start(out=dbg["u000"][:, :], in_=u_ts[0])

                def avbatch(ktb, u_ts):
                    offs = [max(0, (kt - 4 * qc) * P) for kt in (ktb, ktb + 1)]
                    off0 = offs[0]
                    diag = ktb >= 4 * qc
                    tp = ktb // 2
                    for h in pair:
                        if dr_all or not diag:
                            uv = u_ts[h].rearrange("p (j q) -> p j q", j=2)
                            stop_kt = (4 * qc - 2) if not dr_all else (nkt - 2)
                            nc.tensor.matmul(
                                o_ps[h][:, off0:NQ],
                                lhsT=vop_t[tp][:, :, h, :],
                                rhs=uv[:, :, off0:NQ],
                                start=(ktb == 0), stop=(ktb == stop_kt),
                                perf_mode=DR,
                                skip_group_check=True,
                            )
                        else:
                            for j, kt in enumerate((ktb, ktb + 1)):
                                off = offs[j]
                                nc.tensor.matmul(
                                    o_ps[h][:, off:NQ],
                                    lhsT=vo_bf[kt][:, h, :],
                                    rhs=u_ts[h][:, j * NQ + off:(j + 1) * NQ],
                                    start=(kt == 0),
                                    stop=(kt == nkt - 1),
                                    skip_group_check=True,
                                )

                def norm(h):
                    # hw partition_broadcast reads partition 0 regardless of
                    # the input AP offset (and the custom reciprocal is only
                    # reliable at partition 0). Even heads: V sums at PSUM
                    # rows 0:64, denominator at row 64, DMA-shifted to row 0.
                    # Odd heads: denominator already at row 0, V at 64:128,
                    # no shifts, and the mul writes ot rows 64:128 in-lane.
                    m = h // 2
                    # snapshot the accumulator to SBUF first: a [P,NQ] DVE
                    # copy costs the same as a single-row copy (per-lane
                    # throughput) and releases the PSUM buffer for the next
                    # pair immediately instead of after the whole
                    # recip/broadcast chain. The custom reciprocal (and the
                    # gpsimd broadcast source) must sit at partition 0.
                    og = sm.tile([P, NQ], f32, tag="og", name="og")
                    nc.vector.tensor_copy(out=og, in_=o_ps[h])
                    dn = sm.tile([P, NQ], f32, tag="dn", name="dn")
                    rc = sm.tile([P, NQ], f32, tag="rc", name="rc")
                    bc = sm.tile([P, NQ], f32, tag="bc", name="bc")
                    if h % 2 == 0:
                        nc.sync.dma_start(out=dn[0:1, :], in_=og[D:D + 1, :])
                        nc.vector.reciprocal_approx_fast(out=rc[0:1, :], in_=dn[0:1, :])
                        nc.gpsimd.partition_broadcast(bc[0:D, :], rc[0:1, :], channels=D)
                        nc.vector.tensor_mul(
                            ot_t[m][qc][0:D, :], og[0:D, :], bc[0:D, :])
                    else:
                        nc.vector.reciprocal_approx_fast(out=rc[0:1, :], in_=og[0:1, :])
                        # broadcast ignores the output partition offset, so
                        # fill all 128 partitions and use the top half
                        nc.gpsimd.partition_broadcast(bc[:, :], rc[0:1, :], channels=P)
                        nc.vector.tensor_mul(
                            ot_t[m][qc][D:P, :], og[D:P, :], bc[D:P, :])

                alloc()
                for ktb in range(0, nkt, 2):
                    u_ts = {}
                    yield (lambda ktb=ktb, u_ts=u_ts: ebatch(ktb, u_ts))
                    yield (lambda ktb=ktb, u_ts=u_ts: avbatch(ktb, u_ts))
                for h in pair:
                    yield (lambda h=h: norm(h))

            def run_slots(slots, fillers):
                # distribute fillers across the attention slots only (the
                # last two slots are the norms): every filler's engine ops
                # must precede the norm chain, because the norm muls wait on
                # the slow gpsimd broadcast and any PE-feeding DVE op behind
                # them (the next chunk's qt/kt evac) would head-of-line
                # block on the in-order DVE queue
                nf, ns, fi = len(fillers), max(len(slots) - 2, 1), 0
                for i, sf in enumerate(slots):
                    if i >= ns:
                        while fi < nf:
                            fillers[fi]()
                            fi += 1
                    sf()
                    want = min((i + 1) * nf // ns, nf)
                    while fi < want:
                        fillers[fi]()
                        fi += 1

            # emit only what the first attention pair needs up front (heads
            # 0/1 projections + the first two V tiles); the rest interleaves
            # into the first pair's exp-gated gaps
            pc0 = proj_closures(0)
            for f in (pc0[0], pc0[1], pc0[4], pc0[5]):
                f()
            pre = [pc0[6], pc0[7], pc0[2], pc0[3]]
            for qc in range(QC):
                fillers = []
                if qc == 0:
                    fillers += pre
                if qc + 1 < QC:
                    fillers += proj_closures(qc + 1)
                if qc >= 1:
                    fillers += wo_closures(qc - 1)
                s1 = list(pair_stream(qc, (0, 1)))
                s2 = list(pair_stream(qc, (2, 3)))
                half = len(fillers) // 2
                f1, f2 = fillers[:half], fillers[half:]
                if qc == QC - 1:
                    # heads 0/1 are normalized by the end of s1, so the m=0
                    # Wo half hides inside the final pair's attention
                    f2 = f2 + wo_m0_closures(qc)
                run_slots(s1, f1)
                run_slots(s2, f2)
            for f in wo_m1_closures(QC - 1):
                f()
            if debug_taps:
                nc.sync.dma_start(out=dbg["qt00"][:, :], in_=qt_t[0][0])
                nc.sync.dma_start(out=dbg["kt00"][:, :], in_=kt_t[0][0])
                nc.sync.dma_start(out=dbg["vo0"][:, :],
                                  in_=vo_bf[0].rearrange("p h d -> p (h d)"))
                nc.sync.dma_start(out=dbg["ot00"][:, :], in_=ot_t[0][0])
    nc.compile()
    return nc


def _prepare_in_maps(x, Wq, bq, Wk, Wv, Wo):
    import ml_dtypes
    bfd = ml_dtypes.bfloat16
    f8d = ml_dtypes.float8_e4m3fn
    tn = np.tril(np.full((P, P), NEG, np.float32), -1)
    xl = [np.ascontiguousarray(
        x[b].T.reshape(ET, P, T).transpose(1, 0, 2)) for b in range(B)]
    xtb8 = [a.astype(f8d) for a in xl]
    if PH2:
        xtbb = [np.ascontiguousarray(a[:, :, :NQ]).astype(bfd) for a in xl]
    else:
        xtbb = [a.astype(bfd) for a in xl]
    in_maps = []
    for c in range(NCORES):
        b, g = c // G, c % G
        cs = slice(g * CW, (g + 1) * CW)
        bq_g = np.ascontiguousarray(bq[cs].reshape(MT, P).T)
        def wlay(w, dt):  # [E, CW] -> [P, ET, CW] with e = a*P + p
            return np.ascontiguousarray(
                w.reshape(ET, P, CW).transpose(1, 0, 2)).astype(dt)
        wo_l = np.ascontiguousarray(
            Wo[cs, :].reshape(MT, P, E).transpose(1, 0, 2)).astype(bfd)
        in_maps.append({
            "xt": xtb8[b],
            "xtb": xtbb[b],
            "wq": wlay(Wq[:, cs], f8d),
            "wk": wlay(Wk[:, cs], f8d),
            "wv": wlay(Wv[:, cs], bfd),
            "wv8": wlay(Wv[:, cs], f8d),
            "bq": bq_g,
            "wo": wo_l,
            "tn": tn,
        })
    return in_maps


def _run(inputs, trace=False, trace_kwargs=None, debug_taps=False):
    from concourse.bass_utils import run_bass_kernel_spmd

    key = ("nc", debug_taps)
    if key not in _CACHE:
        _CACHE[key] = _build_bass(debug_taps=debug_taps)
    nc = _CACHE[key]

    x = np.asarray(inputs["x"], np.float32)
    Wq = np.asarray(inputs["Wq"], np.float32)
    Wk = np.asarray(inputs["Wk"], np.float32)
    Wv = np.asarray(inputs["Wv"], np.float32)
    Wo = np.asarray(inputs["Wo"], np.float32)
    bq = np.asarray(inputs["bq"], np.float32)
    bv = np.asarray(inputs["bv"], np.float32)
    bo = np.asarray(inputs["bo"], np.float32)

    in_maps = _prepare_in_maps(x, Wq, bq, Wk, Wv, Wo)
    res = run_bass_kernel_spmd(
        nc, in_maps, core_ids=list(range(NCORES)),
        trace=trace, **(trace_kwargs or {}))

    bias_row = (bv @ Wo + bo).astype(np.float32)
    y = np.empty((B, T, E), np.float32)
    for b in range(B):
        acc = res.results[G * b]["out"].astype(np.float32).copy()
        for g in range(1, G):
            acc += res.results[G * b + g]["out"]
        y[b] = acc + bias_row
    return y, res


def kernel(**inputs) -> np.ndarray:
    return _run(inputs, trace=False)[0]
